# revision 10
# baseline (speedup 1.0000x reference)
"""Trainium2 Bass kernel: masked multi-head self-attention block.

out = softmax_mask((x @ Wq) (x @ Wk)^T / sqrt(d)) (x @ Wv) @ Wp + b

Sharding: data-parallel over batch B=8 across the 8 NeuronCores (one
batch row per core); weights replicated; no collectives.

Key compaction: only valid key rows (mask==1) participate; indices are
computed on the host, rows gathered on-device via indirect DMA, padded
to a multiple of 128 (pad slots produce exp()==0).

v3 design (after the latency-bound v2 at 826us):
- Query dim processed in 512-wide quarters; S tiles are [128,2a,512]
  f32 (2 PSUM banks) holding BOTH heads of a pair, so exp runs as one
  [128,1024] instruction per key chunk and the dedicated "s" PSUM tag
  (2 slots) gives 2-key-chunk pipeline depth.  A separate "misc" tag
  serves QKV/proj/transpose/denominator matmuls so they never starve
  the S rotation (the v2 mistake), and "o" (2x 1-bank slots) holds the
  PV accumulators.
- exp engine split: ScalarE for 6 of 9 key chunks, DVE for chunks
  {3,5,7} via the one-op Schraudolph bit-trick (i16 = floor(S*(A/8)+B)
  is the bf16 exp bit pattern); per-partition scale/bias double as the
  pad mask.
- softmax row-sums: bf16 partial adds split DVE/GpSimd by key chunk;
  cross-partition reduction via two accumulating e_r matmuls per
  (quarter, head-pair) into a rotating [2,512] PSUM tile, evacuated to
  a per-quarter [12,512] table; one reciprocal + DRAM broadcast per
  quarter; O~ evacuated unnormalized (bf16) and scaled bf16*bf16.
- x transposes on PE in f32 with 8-wide batched evacuations; the xt
  stream borrows the idle "s" slots during lead-in while xct uses
  "misc", so the two streams run concurrently.
- b_proj is added on the host.
"""
import numpy as np

import concourse.bass as bass
import concourse.tile as tile
from concourse import bacc, mybir
from concourse.bass_utils import run_bass_kernel_spmd
from concourse.masks import make_identity

F32 = mybir.dt.float32
BF16 = mybir.dt.bfloat16
I16 = mybir.dt.int16
I32 = mybir.dt.int32

B, N, DIM = 8, 2048, 768
H, D = 12, 64
SCALE = D ** -0.5
NCH = N // 128        # 16 token chunks
KCH = DIM // 128      # 6 feature chunks
QQ = 4                # query quarters
QW = N // QQ          # 512 queries per quarter
Exp = mybir.ActivationFunctionType.Exp
MUL = mybir.AluOpType.mult
ADD = mybir.AluOpType.add

# Schraudolph constants (floor conversion semantics on DVE)
A16 = 128.0 / float(np.log(2.0))
SCHR_C = 5.1
SCHR_SCALE = A16 * SCALE
SCHR_BIAS = 127.0 * 128.0 - SCHR_C

DVE_EXP_J = (3, 5, 7)     # key chunks whose exp runs on DVE (bit trick)
GPS_RS_J = (1, 6)         # key chunks whose row-sum add runs on GpSimd


def _build(nc, tc, aps, nkc):
    (x_d, ki_d, kb_d, ss_d, sb_d, wqkv_d, wp_d, o_d) = aps
    NK = nkc * 128

    pool = tc.alloc_tile_pool(name="sb", bufs=1)
    ps = tc.alloc_tile_pool(name="ps", bufs=1, space="PSUM")
    dr = tc.alloc_tile_pool(name="dr", bufs=1, space="DRAM")

    def s_ps_tile(shape, name):
        return ps.tile(shape, F32, tag="s", bufs=2, name=name)

    def misc_ps(shape, name):
        return ps.tile(shape, F32, tag="misc", bufs=1, name=name)

    # ---------------- constants ----------------
    ident = pool.tile([128, 128], F32, tag="ident")
    make_identity(nc, ident)
    er = pool.tile([128, 12, 12], BF16, tag="er")
    nc.vector.memset(er, 0.0)
    for r in range(12):
        nc.vector.memset(er[:, r, r:r + 1], 1.0)
    kb_t = pool.tile([128, nkc], F32, tag="kb")
    nc.sync.dma_start(out=kb_t, in_=kb_d.rearrange("(j p) -> p j", p=128))
    ss_t = pool.tile([128, nkc], F32, tag="ss")
    nc.sync.dma_start(out=ss_t, in_=ss_d.rearrange("(j p) -> p j", p=128))
    sb_t = pool.tile([128, nkc], F32, tag="sbv")
    nc.sync.dma_start(out=sb_t, in_=sb_d.rearrange("(j p) -> p j", p=128))
    ki_t = pool.tile([128, nkc], I32, tag="ki")
    nc.sync.dma_start(out=ki_t, in_=ki_d.rearrange("(j p) -> p j", p=128))

    # ---------------- persistent tiles ----------------
    xct = [pool.tile([128, NK], BF16, tag=f"xct{c}", name=f"xct{c}")
           for c in range(KCH)]
    kt = [pool.tile([128, NK], BF16, tag=f"kt{m}", name=f"kt{m}")
          for m in range(KCH)]
    v_nat = [pool.tile([128, DIM], BF16, tag=f"vn{t}", name=f"vn{t}")
             for t in range(nkc)]
    wqk = [pool.tile([128, 2 * DIM], BF16, tag=f"wqk{c}", name=f"wqk{c}")
           for c in range(KCH)]
    wv = [pool.tile([128, DIM], BF16, tag=f"wv{c}", name=f"wv{c}")
          for c in range(KCH)]
    wp = [pool.tile([128, DIM], BF16, tag=f"wp{c}", name=f"wp{c}")
          for c in range(KCH)]

    # ---------------- gathered X_c^T (all gathers first) ------------
    nb = 0
    while nb < nkc:
        bw = min(8, nkc - nb)
        xg_l = []
        for t_i in range(nb, nb + bw):
            xg = pool.tile([128, DIM], F32, tag="xg", bufs=8, name="xg")
            nc.gpsimd.indirect_dma_start(
                out=xg, out_offset=None, in_=x_d,
                in_offset=bass.IndirectOffsetOnAxis(
                    ap=ki_t[:, t_i:t_i + 1], axis=0))
            xg_l.append(xg)
        for c in range(KCH):
            tpb = misc_ps([128, 8, 128], "tpb")
            for i in range(bw):
                nc.tensor.transpose(
                    tpb[:, i, :], xg_l[i][:, c * 128:(c + 1) * 128],
                    ident)
            dst = xct[c][:, nb * 128:(nb + bw) * 128]
            src = tpb.rearrange("p a b -> p (a b)")[:, 0:bw * 128]
            if c % 2 == 0:
                nc.scalar.copy(dst, src)
            else:
                nc.vector.tensor_copy(dst, src)
        nb += bw

    # ---------------- weights: DMA + cast (V first) ----------------
    for c in range(KCH):
        wvs = pool.tile([128, DIM], F32, tag="wvstage", bufs=1,
                        name="wvs")
        nc.sync.dma_start(
            out=wvs, in_=wqkv_d[c * 128:(c + 1) * 128, 2 * DIM:3 * DIM])
        nc.gpsimd.tensor_copy(wv[c], wvs)

    # ---------------- X^T half tiles (8-batched transposes) ---------
    xt_half = {}

    def emit_xt_half(qh):
        xt_tiles = [pool.tile([128, N // 2], BF16, tag=f"xt{c}", bufs=1,
                              name=f"xt{c}h")
                    for c in range(KCH)]
        xt_half[qh] = xt_tiles
        xs = []
        for t in range(qh * 8, qh * 8 + 8):
            x_t = pool.tile([128, DIM], F32, tag="xg", bufs=8, name="x_t")
            nc.sync.dma_start(out=x_t, in_=x_d[t * 128:(t + 1) * 128, :])
            xs.append(x_t)
        for c in range(KCH):
            tpb = s_ps_tile([128, 8, 128], "tpb_s")
            for i in range(8):
                nc.tensor.transpose(
                    tpb[:, i, :], xs[i][:, c * 128:(c + 1) * 128], ident)
            dst = xt_tiles[c]
            src = tpb.rearrange("p a b -> p (a b)")
            if c % 2 == 0:
                nc.vector.tensor_copy(dst, src)
            else:
                nc.scalar.copy(dst, src)

    emit_xt_half(0)

    for c in range(KCH):
        wqs = pool.tile([128, 2 * DIM], F32, tag="wqstage", bufs=1,
                        name="wqs")
        nc.sync.dma_start(
            out=wqs, in_=wqkv_d[c * 128:(c + 1) * 128, 0:2 * DIM])
        nc.vector.tensor_copy(wqk[c], wqs)
        wps = pool.tile([128, DIM], F32, tag="wpstage", bufs=1,
                        name="wps")
        nc.sync.dma_start(out=wps, in_=wp_d[c * 128:(c + 1) * 128, :])
        nc.gpsimd.tensor_copy(wp[c], wps)

    # ---------------- V = X_c @ Wv ----------------
    for t_i in range(nkc):
        v_ps = misc_ps([128, DIM], "v_ps")
        for c in range(KCH):
            nc.tensor.matmul(v_ps[:, 0:512],
                             xct[c][:, t_i * 128:(t_i + 1) * 128],
                             wv[c][:, 0:512],
                             start=(c == 0), stop=(c == KCH - 1))
            nc.tensor.matmul(v_ps[:, 512:DIM],
                             xct[c][:, t_i * 128:(t_i + 1) * 128],
                             wv[c][:, 512:DIM],
                             start=(c == 0), stop=(c == KCH - 1))
        if t_i % 2 == 0:
            nc.scalar.copy(v_nat[t_i], v_ps)
        else:
            nc.vector.tensor_copy(v_nat[t_i], v_ps)

    # ---------------- Q^T (per quarter) and K^T ----------------
    qt_q = [[None] * KCH for _ in range(QQ)]

    def emit_qt(m, qq):
        xt_tiles = xt_half[qq // 2]
        xoff = (qq % 2) * QW
        qtile = pool.tile([128, QW], BF16, tag=f"qt{m}", bufs=2,
                          name=f"qt{m}q")
        qt_q[qq][m] = qtile
        mm_ps = misc_ps([128, QW], "qk_ps")
        for c in range(KCH):
            nc.tensor.matmul(
                mm_ps, wqk[c][:, m * 128:(m + 1) * 128],
                xt_tiles[c][:, xoff:xoff + QW],
                start=(c == 0), stop=(c == KCH - 1))
        if m % 2 == 0:
            nc.scalar.copy(qtile, mm_ps)
        else:
            nc.vector.tensor_copy(qtile, mm_ps)

    def emit_kt(m):
        wcol = DIM + m * 128
        off = 0
        while off < NK:
            w = min(512, NK - off)
            mm_ps = misc_ps([128, QW], "qk_ps")
            for c in range(KCH):
                nc.tensor.matmul(
                    mm_ps[:, 0:w], wqk[c][:, wcol:wcol + 128],
                    xct[c][:, off:off + w],
                    start=(c == 0), stop=(c == KCH - 1))
            if (off // 512) % 2 == 0:
                nc.scalar.copy(kt[m][:, off:off + w], mm_ps[:, 0:w])
            else:
                nc.vector.tensor_copy(kt[m][:, off:off + w],
                                      mm_ps[:, 0:w])
            off += w

    # ---------------- attention for one (qq, hp) ----------------
    def emit_attn(qq, hp, dn_sb):
        kt_c = kt[hp]
        qt_c = qt_q[qq][hp]
        o_ps = ps.tile([128, QW], F32, tag="o", bufs=2, name="o_ps")
        rs = None
        for j in range(nkc):
            s_pair = s_ps_tile([128, 2, QW], "s_pair")
            for a in range(2):
                r0 = a * 64
                nc.tensor.matmul(
                    s_pair[:, a, :],
                    kt_c[r0:r0 + 64, j * 128:(j + 1) * 128],
                    qt_c[r0:r0 + 64, :],
                    start=True, stop=True)
            p_pair = pool.tile([128, 2, QW], BF16, tag="p", bufs=5,
                               name="p_pair")
            s_view = s_pair.rearrange("p a b -> p (a b)")
            p_view = p_pair.rearrange("p a b -> p (a b)")
            if j in DVE_EXP_J:
                nc.vector.tensor_scalar(
                    p_view.bitcast(I16), s_view,
                    ss_t[:, j:j + 1], sb_t[:, j:j + 1], MUL, ADD)
            else:
                nc.scalar.activation(p_view, s_view, Exp,
                                     bias=kb_t[:, j:j + 1], scale=SCALE)
            for a in range(2):
                h = 2 * hp + a
                nc.tensor.matmul(
                    o_ps[a * 64:(a + 1) * 64, :],
                    v_nat[j][:, h * D:(h + 1) * D],
                    p_pair[:, a, :],
                    start=(j == 0), stop=(j == nkc - 1),
                    tile_position=(0, a * 64),
                    skip_group_check=True)
            reng = nc.gpsimd if j in GPS_RS_J else nc.vector
            if j == 0:
                rs = pool.tile([128, 2, QW], BF16, tag="rs", bufs=3,
                               name="rs")
                reng.tensor_copy(rs.rearrange("p a b -> p (a b)"),
                                 p_view)
            else:
                rv = rs.rearrange("p a b -> p (a b)")
                reng.tensor_add(rv, rv, p_view)
        # cross-partition row-sum -> dn_sb rows [2hp, 2hp+2)
        dn_hp = misc_ps([2, QW], "dn_hp")
        for a in range(2):
            nc.tensor.matmul(
                dn_hp, er[:, 2 * hp + a, 2 * hp:2 * hp + 2],
                rs[:, a, :],
                start=(a == 0), stop=(a == 1))
        if hp % 2 == 0:
            nc.scalar.copy(dn_sb[:, hp, :], dn_hp)
        else:
            nc.vector.tensor_copy(dn_sb[:, hp, :], dn_hp)
        # evacuate unnormalized O~ (bf16); normalize later
        oe = pool.tile([128, QW], BF16, tag="oe", bufs=7, name="oe")
        if hp % 2 == 0:
            nc.vector.tensor_copy(oe, o_ps)
        else:
            nc.scalar.copy(oe, o_ps)
        return oe

    # ---------------- per-quarter tail ----------------
    ot = [[None] * QQ for _ in range(KCH)]

    def emit_qq_tail(qq, oe_l, dn_sb):
        # reshuffle [2a, 6hp, 512] -> [12, 512] rows (2hp+a) via DRAM
        dn_dram = dr.tile([12, QW], F32, tag="dn_dram", bufs=2,
                          name="dn_dram")
        nc.sync.dma_start(
            out=dn_dram.rearrange("(h a) q -> a h q", a=2), in_=dn_sb)
        dn_12 = pool.tile([12, QW], F32, tag="dn12", bufs=2, name="dn_12")
        nc.sync.dma_start(out=dn_12, in_=dn_dram)
        rc_sb = pool.tile([12, QW], F32, tag="rc_sb", bufs=2, name="rc_sb")
        nc.vector.reciprocal_approx_fast(out=rc_sb, in_=dn_12)
        rc_bf = pool.tile([12, QW], BF16, tag="rc_bf", bufs=2, name="rc_bf")
        nc.vector.tensor_copy(rc_bf, rc_sb)
        rc_dram = dr.tile([12, QW], BF16, tag="rc_dram", bufs=2,
                          name="rc_dram")
        nc.sync.dma_start(out=rc_dram, in_=rc_bf)
        for hp in range(KCH):
            rc_bc = pool.tile([128, QW], BF16, tag="rc_bc", bufs=3,
                              name="rc_bc")
            for a in range(2):
                row = rc_dram[2 * hp + a:2 * hp + a + 1, :]
                bc_ap = bass.AP(
                    tensor=row.tensor, offset=row.offset,
                    ap=[[0, 64]] + [list(p) for p in row.ap[1:]])
                nc.sync.dma_start(out=rc_bc[a * 64:(a + 1) * 64, :],
                                  in_=bc_ap)
            ott = pool.tile([128, QW], BF16, tag="ot", bufs=7, name="ott")
            nc.vector.tensor_mul(ott, oe_l[hp], rc_bc)
            ot[hp][qq] = ott
        tq = NCH // QQ
        for t_i in range(qq * tq, (qq + 1) * tq):
            tl = (t_i % tq) * 128
            pr_ps = misc_ps([128, 2, 512], "pr_ps")
            for c in range(KCH):
                nc.tensor.matmul(
                    pr_ps[:, 0, :], ot[c][qq][:, tl:tl + 128],
                    wp[c][:, 0:512],
                    start=(c == 0), stop=(c == KCH - 1))
                nc.tensor.matmul(
                    pr_ps[:, 1, 0:256], ot[c][qq][:, tl:tl + 128],
                    wp[c][:, 512:DIM],
                    start=(c == 0), stop=(c == KCH - 1))
            out_t = pool.tile([128, DIM], F32, tag="out_t", bufs=2,
                              name="out_t")
            if t_i % 2 == 0:
                nc.scalar.copy(
                    out_t, pr_ps.rearrange("p a b -> p (a b)")[:, 0:DIM])
            else:
                nc.vector.tensor_copy(
                    out_t, pr_ps.rearrange("p a b -> p (a b)")[:, 0:DIM])
            nc.sync.dma_start(
                out=o_d[t_i * 128:(t_i + 1) * 128, :], in_=out_t)

    # ---------------- wavefront emission ----------------
    for qq in range(QQ):
        if qq == 2:
            emit_xt_half(1)
        dn_sb = pool.tile([2, KCH, QW], F32, tag="dn_sb", bufs=1,
                          name="dn_sb")
        oe_l = []
        for hp in range(KCH):
            if qq == 0:
                emit_kt(hp)
            emit_qt(hp, qq)
            oe_l.append(emit_attn(qq, hp, dn_sb))
        emit_qq_tail(qq, oe_l, dn_sb)

    pool.release()
    ps.release()
    dr.release()


_CACHE = {}


def _get_compiled(nkc):
    if nkc in _CACHE:
        return _CACHE[nkc]
    NK = nkc * 128
    nc = bacc.Bacc("TRN2", target_bir_lowering=False, debug=False,
                   num_devices=B)
    x_d = nc.dram_tensor("x", [N, DIM], F32, kind="ExternalInput").ap()
    ki_d = nc.dram_tensor("kidx", [NK], I32, kind="ExternalInput").ap()
    kb_d = nc.dram_tensor("kbias", [NK], F32, kind="ExternalInput").ap()
    ss_d = nc.dram_tensor("sscale", [NK], F32, kind="ExternalInput").ap()
    sb_d = nc.dram_tensor("sbias", [NK], F32, kind="ExternalInput").ap()
    wqkv_d = nc.dram_tensor("w_qkv", [DIM, 3 * DIM], F32,
                            kind="ExternalInput").ap()
    wp_d = nc.dram_tensor("w_proj", [DIM, DIM], F32,
                          kind="ExternalInput").ap()
    o_d = nc.dram_tensor("out", [N, DIM], F32, kind="ExternalOutput").ap()
    with tile.TileContext(nc) as tc:
        _build(nc, tc, (x_d, ki_d, kb_d, ss_d, sb_d, wqkv_d, wp_d, o_d),
               nkc)
    nc.compile()
    _CACHE[nkc] = nc
    return nc


def prep_run(x, mask, w_qkv, w_proj, b_proj):
    x = np.ascontiguousarray(np.asarray(x, dtype=np.float32))
    mask = np.ascontiguousarray(np.asarray(mask, dtype=np.int32))
    w_qkv = np.ascontiguousarray(np.asarray(w_qkv, dtype=np.float32))
    w_proj = np.ascontiguousarray(np.asarray(w_proj, dtype=np.float32))

    idxs = [np.flatnonzero(mask[b]).astype(np.int32) for b in range(B)]
    max_valid = max(len(i) for i in idxs)
    nkc = min(NCH, max(1, -(-max_valid // 128)))
    NK = nkc * 128
    kidx = np.zeros((B, NK), dtype=np.int32)
    kbias = np.full((B, NK), -1.0e30, dtype=np.float32)
    sscale = np.zeros((B, NK), dtype=np.float32)
    sbias = np.zeros((B, NK), dtype=np.float32)
    for b in range(B):
        n = len(idxs[b])
        kidx[b, :n] = idxs[b]
        kbias[b, :n] = 0.0
        sscale[b, :n] = SCHR_SCALE
        sbias[b, :n] = SCHR_BIAS

    nc = _get_compiled(nkc)
    in_maps = [
        {"x": x[b], "kidx": kidx[b], "kbias": kbias[b],
         "sscale": sscale[b], "sbias": sbias[b],
         "w_qkv": w_qkv, "w_proj": w_proj}
        for b in range(B)
    ]
    return nc, in_maps


def kernel(x, mask, w_qkv, w_proj, b_proj):
    nc, in_maps = prep_run(x, mask, w_qkv, w_proj, b_proj)
    b_proj = np.asarray(b_proj, dtype=np.float32)
    last_err = None
    for _ in range(3):
        try:
            res = run_bass_kernel_spmd(nc, in_maps, list(range(B))).results
            out = np.stack([res[b]["out"] for b in range(B)], axis=0)
            return out + b_proj
        except Exception as e:  # transient device hiccup: retry
            last_err = e
    raise last_err


# revision 12
# speedup vs baseline: 1.6042x; 1.6042x over previous
"""Trainium2 Bass kernel: masked multi-head self-attention block.

out = softmax_mask((x @ Wq) (x @ Wk)^T / sqrt(d)) (x @ Wv) @ Wp + b

Sharding: data-parallel over batch B=8 across the 8 NeuronCores (one
batch row per core); weights replicated; no collectives.

Key compaction: only valid key rows (mask==1) participate; indices are
computed on the host, rows gathered on-device via indirect DMA, padded
to a multiple of 128 (pad slots produce exp()==0).

v3 design (after the latency-bound v2 at 826us):
- Query dim processed in 512-wide quarters; S tiles are [128,2a,512]
  f32 (2 PSUM banks) holding BOTH heads of a pair, so exp runs as one
  [128,1024] instruction per key chunk and the dedicated "s" PSUM tag
  (2 slots) gives 2-key-chunk pipeline depth.  A separate "misc" tag
  serves QKV/proj/transpose/denominator matmuls so they never starve
  the S rotation (the v2 mistake), and "o" (2x 1-bank slots) holds the
  PV accumulators.
- exp engine split: ScalarE for 6 of 9 key chunks, DVE for chunks
  {3,5,7} via the one-op Schraudolph bit-trick (i16 = floor(S*(A/8)+B)
  is the bf16 exp bit pattern); per-partition scale/bias double as the
  pad mask.
- softmax row-sums: bf16 partial adds split DVE/GpSimd by key chunk;
  cross-partition reduction via two accumulating e_r matmuls per
  (quarter, head-pair) into a rotating [2,512] PSUM tile, evacuated to
  a per-quarter [12,512] table; one reciprocal + DRAM broadcast per
  quarter; O~ evacuated unnormalized (bf16) and scaled bf16*bf16.
- x transposes on PE in f32 with 8-wide batched evacuations; the xt
  stream borrows the idle "s" slots during lead-in while xct uses
  "misc", so the two streams run concurrently.
- b_proj is added on the host.
"""
import numpy as np

import concourse.bass as bass
import concourse.tile as tile
from concourse import bacc, mybir
from concourse.bass_utils import run_bass_kernel_spmd
from concourse.masks import make_identity

F32 = mybir.dt.float32
BF16 = mybir.dt.bfloat16
I16 = mybir.dt.int16
I32 = mybir.dt.int32

B, N, DIM = 8, 2048, 768
H, D = 12, 64
SCALE = D ** -0.5
NCH = N // 128        # 16 token chunks
KCH = DIM // 128      # 6 feature chunks
QQ = 4                # query quarters
QW = N // QQ          # 512 queries per quarter
Exp = mybir.ActivationFunctionType.Exp
MUL = mybir.AluOpType.mult
ADD = mybir.AluOpType.add

# Schraudolph constants (floor conversion semantics on DVE)
A16 = 128.0 / float(np.log(2.0))
SCHR_C = 5.1
SCHR_SCALE = A16 * SCALE
SCHR_BIAS = 127.0 * 128.0 - SCHR_C

def dve_exp(j, hp):
    return j == 5 or (j == 7 and hp % 2 == 0)


def gps_rs(j, hp):
    return j in (1, 6) or (j == 4 and hp % 2 == 0)


def _build(nc, tc, aps, nkc):
    (x_d, ki_d, kb_d, ss_d, sb_d, wqkv_d, wp_d, o_d) = aps
    NK = nkc * 128

    pool = tc.alloc_tile_pool(name="sb", bufs=1)
    ps = tc.alloc_tile_pool(name="ps", bufs=1, space="PSUM")
    dr = tc.alloc_tile_pool(name="dr", bufs=1, space="DRAM")

    def s_ps_tile(shape, name):
        return ps.tile(shape, F32, tag="s", bufs=2, name=name)

    def misc_ps(shape, name):
        return ps.tile(shape, F32, tag="misc", bufs=1, name=name)

    # ---------------- constants ----------------
    ident = pool.tile([128, 128], F32, tag="ident")
    make_identity(nc, ident)
    er = pool.tile([128, 12, 12], BF16, tag="er")
    nc.vector.memset(er, 0.0)
    for r in range(12):
        nc.vector.memset(er[:, r, r:r + 1], 1.0)
    kb_t = pool.tile([128, nkc], F32, tag="kb")
    nc.sync.dma_start(out=kb_t, in_=kb_d.rearrange("(j p) -> p j", p=128))
    ss_t = pool.tile([128, nkc], F32, tag="ss")
    nc.sync.dma_start(out=ss_t, in_=ss_d.rearrange("(j p) -> p j", p=128))
    sb_t = pool.tile([128, nkc], F32, tag="sbv")
    nc.sync.dma_start(out=sb_t, in_=sb_d.rearrange("(j p) -> p j", p=128))
    ki_t = pool.tile([128, nkc], I32, tag="ki")
    nc.sync.dma_start(out=ki_t, in_=ki_d.rearrange("(j p) -> p j", p=128))

    # ---------------- persistent tiles ----------------
    xct = [pool.tile([128, NK], BF16, tag=f"xct{c}", name=f"xct{c}")
           for c in range(KCH)]
    kt = [pool.tile([128, NK], BF16, tag=f"kt{m}", name=f"kt{m}")
          for m in range(KCH)]
    v_nat = [pool.tile([128, DIM], BF16, tag=f"vn{t}", name=f"vn{t}")
             for t in range(nkc)]
    wqk = [pool.tile([128, 2 * DIM], BF16, tag=f"wqk{c}", name=f"wqk{c}")
           for c in range(KCH)]
    wv = [pool.tile([128, DIM], BF16, tag=f"wv{c}", name=f"wv{c}")
          for c in range(KCH)]
    wp = [pool.tile([128, DIM], BF16, tag=f"wp{c}", name=f"wp{c}")
          for c in range(KCH)]

    # ---------------- gathered X_c^T (all gathers first) ------------
    nb = 0
    while nb < nkc:
        bw = min(8, nkc - nb)
        xg_l = []
        for t_i in range(nb, nb + bw):
            xg = pool.tile([128, DIM], F32, tag="xg", bufs=8, name="xg")
            nc.gpsimd.indirect_dma_start(
                out=xg, out_offset=None, in_=x_d,
                in_offset=bass.IndirectOffsetOnAxis(
                    ap=ki_t[:, t_i:t_i + 1], axis=0))
            xg_l.append(xg)
        for c in range(KCH):
            tpb = misc_ps([128, 8, 128], "tpb")
            for i in range(bw):
                nc.tensor.transpose(
                    tpb[:, i, :], xg_l[i][:, c * 128:(c + 1) * 128],
                    ident)
            dst = xct[c][:, nb * 128:(nb + bw) * 128]
            src = tpb.rearrange("p a b -> p (a b)")[:, 0:bw * 128]
            if c % 2 == 0:
                nc.scalar.copy(dst, src)
            else:
                nc.vector.tensor_copy(dst, src)
        nb += bw

    # ---------------- weights: DMA + cast (V first) ----------------
    for c in range(KCH):
        wvs = pool.tile([128, DIM], F32, tag="wvstage", bufs=1,
                        name="wvs")
        nc.sync.dma_start(
            out=wvs, in_=wqkv_d[c * 128:(c + 1) * 128, 2 * DIM:3 * DIM])
        nc.gpsimd.tensor_copy(wv[c], wvs)

    # ---------------- X^T half tiles (8-batched transposes) ---------
    xt_half = {}

    def emit_xt_half(qh):
        xt_tiles = [pool.tile([128, N // 2], BF16, tag=f"xt{c}", bufs=1,
                              name=f"xt{c}h")
                    for c in range(KCH)]
        xt_half[qh] = xt_tiles
        xs = []
        for t in range(qh * 8, qh * 8 + 8):
            x_t = pool.tile([128, DIM], F32, tag="xg", bufs=8, name="x_t")
            nc.sync.dma_start(out=x_t, in_=x_d[t * 128:(t + 1) * 128, :])
            xs.append(x_t)
        for c in range(KCH):
            tpb = s_ps_tile([128, 8, 128], "tpb_s")
            for i in range(8):
                nc.tensor.transpose(
                    tpb[:, i, :], xs[i][:, c * 128:(c + 1) * 128], ident)
            dst = xt_tiles[c]
            src = tpb.rearrange("p a b -> p (a b)")
            if c % 2 == 0:
                nc.vector.tensor_copy(dst, src)
            else:
                nc.scalar.copy(dst, src)

    emit_xt_half(0)

    for c in range(KCH):
        wqs = pool.tile([128, 2 * DIM], F32, tag="wqstage", bufs=1,
                        name="wqs")
        nc.sync.dma_start(
            out=wqs, in_=wqkv_d[c * 128:(c + 1) * 128, 0:2 * DIM])
        nc.vector.tensor_copy(wqk[c], wqs)
        wps = pool.tile([128, DIM], F32, tag="wpstage", bufs=1,
                        name="wps")
        nc.sync.dma_start(out=wps, in_=wp_d[c * 128:(c + 1) * 128, :])
        nc.gpsimd.tensor_copy(wp[c], wps)

    # ---------------- V = X_c @ Wv ----------------
    for t_i in range(nkc):
        v_ps = misc_ps([128, DIM], "v_ps")
        for c in range(KCH):
            nc.tensor.matmul(v_ps[:, 0:512],
                             xct[c][:, t_i * 128:(t_i + 1) * 128],
                             wv[c][:, 0:512],
                             start=(c == 0), stop=(c == KCH - 1))
            nc.tensor.matmul(v_ps[:, 512:DIM],
                             xct[c][:, t_i * 128:(t_i + 1) * 128],
                             wv[c][:, 512:DIM],
                             start=(c == 0), stop=(c == KCH - 1))
        if t_i % 2 == 0:
            nc.scalar.copy(v_nat[t_i], v_ps)
        else:
            nc.vector.tensor_copy(v_nat[t_i], v_ps)

    # ---------------- Q^T (per quarter) and K^T ----------------
    qt_q = [[None] * KCH for _ in range(QQ)]

    def emit_qt(m, qq):
        xt_tiles = xt_half[qq // 2]
        xoff = (qq % 2) * QW
        qtile = pool.tile([128, QW], BF16, tag=f"qt{m}", bufs=2,
                          name=f"qt{m}q")
        qt_q[qq][m] = qtile
        mm_ps = misc_ps([128, QW], "qk_ps")
        for c in range(KCH):
            nc.tensor.matmul(
                mm_ps, wqk[c][:, m * 128:(m + 1) * 128],
                xt_tiles[c][:, xoff:xoff + QW],
                start=(c == 0), stop=(c == KCH - 1))
        if m % 2 == 0:
            nc.scalar.copy(qtile, mm_ps)
        else:
            nc.vector.tensor_copy(qtile, mm_ps)

    def emit_kt(m):
        wcol = DIM + m * 128
        off = 0
        while off < NK:
            w = min(512, NK - off)
            mm_ps = misc_ps([128, QW], "qk_ps")
            for c in range(KCH):
                nc.tensor.matmul(
                    mm_ps[:, 0:w], wqk[c][:, wcol:wcol + 128],
                    xct[c][:, off:off + w],
                    start=(c == 0), stop=(c == KCH - 1))
            if (off // 512) % 2 == 0:
                nc.scalar.copy(kt[m][:, off:off + w], mm_ps[:, 0:w])
            else:
                nc.vector.tensor_copy(kt[m][:, off:off + w],
                                      mm_ps[:, 0:w])
            off += w

    # ---------------- attention for one (qq, hp) ----------------
    def emit_attn(qq, hp, rc_sb, close_prev):
        kt_c = kt[hp]
        qt_c = qt_q[qq][hp]
        o_ps = ps.tile([128, QW], F32, tag="o", bufs=2, name="o_ps")
        s_tiles = {}

        def emit_s(j):
            sp = s_ps_tile([128, 2, QW], "s_pair")
            for a in range(2):
                r0 = a * 64
                nc.tensor.matmul(
                    sp[:, a, :],
                    kt_c[r0:r0 + 64, j * 128:(j + 1) * 128],
                    qt_c[r0:r0 + 64, :],
                    start=True, stop=True)
            s_tiles[j] = sp

        emit_s(0)
        if close_prev is not None:
            close_prev()
        if nkc > 1:
            emit_s(1)
        rs = pool.tile([128, 2, QW], BF16, tag="rs", bufs=3, name="rs")
        rv = rs.rearrange("p a b -> p (a b)")
        for j in range(nkc):
            s_pair = s_tiles.pop(j)
            p_pair = pool.tile([128, 2, QW], BF16, tag="p", bufs=5,
                               name="p_pair")
            s_view = s_pair.rearrange("p a b -> p (a b)")
            p_view = p_pair.rearrange("p a b -> p (a b)")
            if dve_exp(j, hp):
                nc.vector.tensor_scalar(
                    p_view.bitcast(I16), s_view,
                    ss_t[:, j:j + 1], sb_t[:, j:j + 1], MUL, ADD)
            else:
                nc.scalar.activation(p_view, s_view, Exp,
                                     bias=kb_t[:, j:j + 1], scale=SCALE)
            for a in range(2):
                h = 2 * hp + a
                nc.tensor.matmul(
                    o_ps[a * 64:(a + 1) * 64, :],
                    v_nat[j][:, h * D:(h + 1) * D],
                    p_pair[:, a, :],
                    start=(j == 0), stop=(j == nkc - 1),
                    tile_position=(0, a * 64),
                    skip_group_check=True)
            if j + 2 < nkc:
                emit_s(j + 2)
            reng = nc.gpsimd if gps_rs(j, hp) else nc.vector
            if j == 0:
                reng.tensor_copy(rv, p_view)
            else:
                reng.tensor_add(rv, rv, p_view)
        # evacuate unnormalized O~ (bf16); normalize later
        oe = pool.tile([128, QW], BF16, tag="oe", bufs=7, name="oe")
        if hp % 2 == 0:
            nc.vector.tensor_copy(oe, o_ps)
        else:
            nc.scalar.copy(oe, o_ps)

        def close():
            dn_hp = misc_ps([2, QW], "dn_hp")
            for a in range(2):
                nc.tensor.matmul(
                    dn_hp, er[:, 2 * hp + a, 2 * hp:2 * hp + 2],
                    rs[:, a, :],
                    start=(a == 0), stop=(a == 1))
            nc.vector.reciprocal_approx_fast(
                out=rc_sb[:, hp, :], in_=dn_hp)

        return oe, close

    # ---------------- per-quarter tail ----------------
    ot = [[None] * QQ for _ in range(KCH)]

    def emit_qq_tail(qq, oe_l, rc_sb):
        rc_dram = dr.tile([2, KCH, QW], F32, tag="rc_dram", bufs=2,
                          name="rc_dram")
        nc.sync.dma_start(out=rc_dram, in_=rc_sb)
        for hp in range(KCH):
            rc_bc = pool.tile([128, QW], F32, tag="rc_bc", bufs=3,
                              name="rc_bc")
            for a in range(2):
                row = rc_dram[a:a + 1, hp:hp + 1, :]
                bc_ap = bass.AP(
                    tensor=row.tensor, offset=row.offset,
                    ap=[[0, 64]] + [list(p) for p in row.ap[1:]])
                nc.sync.dma_start(out=rc_bc[a * 64:(a + 1) * 64, :],
                                  in_=bc_ap)
            ott = pool.tile([128, QW], BF16, tag="ot", bufs=7, name="ott")
            nc.vector.tensor_mul(ott, oe_l[hp], rc_bc)
            ot[hp][qq] = ott
        tq = NCH // QQ
        for t_i in range(qq * tq, (qq + 1) * tq):
            tl = (t_i % tq) * 128
            pr_ps = misc_ps([128, 2, 512], "pr_ps")
            for c in range(KCH):
                nc.tensor.matmul(
                    pr_ps[:, 0, :], ot[c][qq][:, tl:tl + 128],
                    wp[c][:, 0:512],
                    start=(c == 0), stop=(c == KCH - 1))
                nc.tensor.matmul(
                    pr_ps[:, 1, 0:256], ot[c][qq][:, tl:tl + 128],
                    wp[c][:, 512:DIM],
                    start=(c == 0), stop=(c == KCH - 1))
            out_t = pool.tile([128, DIM], F32, tag="out_t", bufs=2,
                              name="out_t")
            if t_i % 2 == 0:
                nc.scalar.copy(
                    out_t, pr_ps.rearrange("p a b -> p (a b)")[:, 0:DIM])
            else:
                nc.vector.tensor_copy(
                    out_t, pr_ps.rearrange("p a b -> p (a b)")[:, 0:DIM])
            nc.sync.dma_start(
                out=o_d[t_i * 128:(t_i + 1) * 128, :], in_=out_t)

    # ---------------- wavefront emission ----------------
    for m in range(KCH):
        emit_kt(m)
    for m in range(KCH):
        emit_qt(m, 0)
    for qq in range(QQ):
        rc_sb = pool.tile([2, KCH, QW], F32, tag="rc_sb", bufs=1,
                          name="rc_sb")
        oe_l = []
        close = None
        for hp in range(KCH):
            oe, close = emit_attn(qq, hp, rc_sb, close)
            oe_l.append(oe)
        close()
        if qq == 1:
            emit_xt_half(1)
        if qq + 1 < QQ:
            for m in range(KCH):
                emit_qt(m, qq + 1)
        emit_qq_tail(qq, oe_l, rc_sb)

    pool.release()
    ps.release()
    dr.release()


_CACHE = {}


def _get_compiled(nkc):
    if nkc in _CACHE:
        return _CACHE[nkc]
    NK = nkc * 128
    nc = bacc.Bacc("TRN2", target_bir_lowering=False, debug=False,
                   num_devices=B)
    x_d = nc.dram_tensor("x", [N, DIM], F32, kind="ExternalInput").ap()
    ki_d = nc.dram_tensor("kidx", [NK], I32, kind="ExternalInput").ap()
    kb_d = nc.dram_tensor("kbias", [NK], F32, kind="ExternalInput").ap()
    ss_d = nc.dram_tensor("sscale", [NK], F32, kind="ExternalInput").ap()
    sb_d = nc.dram_tensor("sbias", [NK], F32, kind="ExternalInput").ap()
    wqkv_d = nc.dram_tensor("w_qkv", [DIM, 3 * DIM], F32,
                            kind="ExternalInput").ap()
    wp_d = nc.dram_tensor("w_proj", [DIM, DIM], F32,
                          kind="ExternalInput").ap()
    o_d = nc.dram_tensor("out", [N, DIM], F32, kind="ExternalOutput").ap()
    with tile.TileContext(nc) as tc:
        _build(nc, tc, (x_d, ki_d, kb_d, ss_d, sb_d, wqkv_d, wp_d, o_d),
               nkc)
    nc.compile()
    _CACHE[nkc] = nc
    return nc


def prep_run(x, mask, w_qkv, w_proj, b_proj):
    x = np.ascontiguousarray(np.asarray(x, dtype=np.float32))
    mask = np.ascontiguousarray(np.asarray(mask, dtype=np.int32))
    w_qkv = np.ascontiguousarray(np.asarray(w_qkv, dtype=np.float32))
    w_proj = np.ascontiguousarray(np.asarray(w_proj, dtype=np.float32))

    idxs = [np.flatnonzero(mask[b]).astype(np.int32) for b in range(B)]
    max_valid = max(len(i) for i in idxs)
    nkc = min(NCH, max(1, -(-max_valid // 128)))
    NK = nkc * 128
    kidx = np.zeros((B, NK), dtype=np.int32)
    kbias = np.full((B, NK), -1.0e30, dtype=np.float32)
    sscale = np.zeros((B, NK), dtype=np.float32)
    sbias = np.zeros((B, NK), dtype=np.float32)
    for b in range(B):
        n = len(idxs[b])
        kidx[b, :n] = idxs[b]
        kbias[b, :n] = 0.0
        sscale[b, :n] = SCHR_SCALE
        sbias[b, :n] = SCHR_BIAS

    nc = _get_compiled(nkc)
    in_maps = [
        {"x": x[b], "kidx": kidx[b], "kbias": kbias[b],
         "sscale": sscale[b], "sbias": sbias[b],
         "w_qkv": w_qkv, "w_proj": w_proj}
        for b in range(B)
    ]
    return nc, in_maps


def kernel(x, mask, w_qkv, w_proj, b_proj):
    nc, in_maps = prep_run(x, mask, w_qkv, w_proj, b_proj)
    b_proj = np.asarray(b_proj, dtype=np.float32)
    last_err = None
    for _ in range(3):
        try:
            res = run_bass_kernel_spmd(nc, in_maps, list(range(B))).results
            out = np.stack([res[b]["out"] for b in range(B)], axis=0)
            return out + b_proj
        except Exception as e:  # transient device hiccup: retry
            last_err = e
    raise last_err


# revision 13
# speedup vs baseline: 1.7043x; 1.0624x over previous
"""Trainium2 Bass kernel: masked multi-head self-attention block.

out = softmax_mask((x @ Wq) (x @ Wk)^T / sqrt(d)) (x @ Wv) @ Wp + b

Sharding: data-parallel over batch B=8 across the 8 NeuronCores (one
batch row per core); weights replicated; no collectives.

Key compaction: only valid key rows (mask==1) participate; indices are
computed on the host, rows gathered on-device via indirect DMA, padded
to a multiple of 128 (pad slots produce exp()==0).

v3 design (after the latency-bound v2 at 826us):
- Query dim processed in 512-wide quarters; S tiles are [128,2a,512]
  f32 (2 PSUM banks) holding BOTH heads of a pair, so exp runs as one
  [128,1024] instruction per key chunk and the dedicated "s" PSUM tag
  (2 slots) gives 2-key-chunk pipeline depth.  A separate "misc" tag
  serves QKV/proj/transpose/denominator matmuls so they never starve
  the S rotation (the v2 mistake), and "o" (2x 1-bank slots) holds the
  PV accumulators.
- exp engine split: ScalarE for 6 of 9 key chunks, DVE for chunks
  {3,5,7} via the one-op Schraudolph bit-trick (i16 = floor(S*(A/8)+B)
  is the bf16 exp bit pattern); per-partition scale/bias double as the
  pad mask.
- softmax row-sums: bf16 partial adds split DVE/GpSimd by key chunk;
  cross-partition reduction via two accumulating e_r matmuls per
  (quarter, head-pair) into a rotating [2,512] PSUM tile, evacuated to
  a per-quarter [12,512] table; one reciprocal + DRAM broadcast per
  quarter; O~ evacuated unnormalized (bf16) and scaled bf16*bf16.
- x transposes on PE in f32 with 8-wide batched evacuations; the xt
  stream borrows the idle "s" slots during lead-in while xct uses
  "misc", so the two streams run concurrently.
- b_proj is added on the host.
"""
import numpy as np

import concourse.bass as bass
import concourse.tile as tile
from concourse import bacc, mybir
from concourse.bass_utils import run_bass_kernel_spmd
from concourse.masks import make_identity

F32 = mybir.dt.float32
BF16 = mybir.dt.bfloat16
I16 = mybir.dt.int16
I32 = mybir.dt.int32

B, N, DIM = 8, 2048, 768
H, D = 12, 64
SCALE = D ** -0.5
NCH = N // 128        # 16 token chunks
KCH = DIM // 128      # 6 feature chunks
QQ = 4                # query quarters
QW = N // QQ          # 512 queries per quarter
Exp = mybir.ActivationFunctionType.Exp
MUL = mybir.AluOpType.mult
ADD = mybir.AluOpType.add

# Schraudolph constants (floor conversion semantics on DVE)
A16 = 128.0 / float(np.log(2.0))
SCHR_C = 5.1
SCHR_SCALE = A16 * SCALE
SCHR_BIAS = 127.0 * 128.0 - SCHR_C

def dve_exp(j, hp):
    return j == 5 or (j == 7 and hp % 2 == 0)


def gps_rs(j, hp):
    return j in (1, 4, 6)


def _build(nc, tc, aps, nkc):
    (x_d, ki_d, kb_d, ss_d, sb_d, wqkv_d, wp_d, o_d) = aps
    NK = nkc * 128

    pool = tc.alloc_tile_pool(name="sb", bufs=1)
    ps = tc.alloc_tile_pool(name="ps", bufs=1, space="PSUM")
    dr = tc.alloc_tile_pool(name="dr", bufs=1, space="DRAM")

    def s_ps_tile(shape, name):
        return ps.tile(shape, F32, tag="s", bufs=2, name=name)

    def misc_ps(shape, name):
        return ps.tile(shape, F32, tag="misc", bufs=1, name=name)

    # ---------------- constants ----------------
    ident = pool.tile([128, 128], F32, tag="ident")
    make_identity(nc, ident)
    er = pool.tile([128, 12, 12], BF16, tag="er")
    nc.vector.memset(er, 0.0)
    for r in range(12):
        nc.vector.memset(er[:, r, r:r + 1], 1.0)
    kb_t = pool.tile([128, nkc], F32, tag="kb")
    nc.sync.dma_start(out=kb_t, in_=kb_d.rearrange("(j p) -> p j", p=128))
    ss_t = pool.tile([128, nkc], F32, tag="ss")
    nc.sync.dma_start(out=ss_t, in_=ss_d.rearrange("(j p) -> p j", p=128))
    sb_t = pool.tile([128, nkc], F32, tag="sbv")
    nc.sync.dma_start(out=sb_t, in_=sb_d.rearrange("(j p) -> p j", p=128))
    ki_t = pool.tile([128, nkc], I32, tag="ki")
    nc.sync.dma_start(out=ki_t, in_=ki_d.rearrange("(j p) -> p j", p=128))

    # ---------------- persistent tiles ----------------
    xct = [pool.tile([128, NK], BF16, tag=f"xct{c}", name=f"xct{c}")
           for c in range(KCH)]
    kt = [pool.tile([128, NK], BF16, tag=f"kt{m}", name=f"kt{m}")
          for m in range(KCH)]
    v_nat = [pool.tile([128, DIM], BF16, tag=f"vn{t}", name=f"vn{t}")
             for t in range(nkc)]
    wqk = [pool.tile([128, 2 * DIM], BF16, tag=f"wqk{c}", name=f"wqk{c}")
           for c in range(KCH)]
    wv = [pool.tile([128, DIM], BF16, tag=f"wv{c}", name=f"wv{c}")
          for c in range(KCH)]
    wp = [pool.tile([128, DIM], BF16, tag=f"wp{c}", name=f"wp{c}")
          for c in range(KCH)]

    # ---------------- gathered X_c^T (all gathers first) ------------
    nb = 0
    while nb < nkc:
        bw = min(8, nkc - nb)
        xg_l = []
        for t_i in range(nb, nb + bw):
            xg = pool.tile([128, DIM], F32, tag="xg", bufs=8, name="xg")
            nc.gpsimd.indirect_dma_start(
                out=xg, out_offset=None, in_=x_d,
                in_offset=bass.IndirectOffsetOnAxis(
                    ap=ki_t[:, t_i:t_i + 1], axis=0))
            xg_l.append(xg)
        for c in range(KCH):
            tpb = misc_ps([128, 8, 128], "tpb")
            for i in range(bw):
                nc.tensor.transpose(
                    tpb[:, i, :], xg_l[i][:, c * 128:(c + 1) * 128],
                    ident)
            dst = xct[c][:, nb * 128:(nb + bw) * 128]
            src = tpb.rearrange("p a b -> p (a b)")[:, 0:bw * 128]
            if c % 2 == 0:
                nc.scalar.copy(dst, src)
            else:
                nc.vector.tensor_copy(dst, src)
        nb += bw

    # ---------------- weights: DMA + cast (V first) ----------------
    for c in range(KCH):
        wvs = pool.tile([128, DIM], F32, tag="wvstage", bufs=1,
                        name="wvs")
        nc.sync.dma_start(
            out=wvs, in_=wqkv_d[c * 128:(c + 1) * 128, 2 * DIM:3 * DIM])
        nc.gpsimd.tensor_copy(wv[c], wvs)

    # ---------------- X^T half tiles (8-batched transposes) ---------
    xt_half = {}

    def emit_xt_half(qh):
        xt_tiles = [pool.tile([128, N // 2], BF16, tag=f"xt{c}", bufs=1,
                              name=f"xt{c}h")
                    for c in range(KCH)]
        xt_half[qh] = xt_tiles
        xs = []
        for t in range(qh * 8, qh * 8 + 8):
            x_t = pool.tile([128, DIM], F32, tag="xg", bufs=8, name="x_t")
            nc.sync.dma_start(out=x_t, in_=x_d[t * 128:(t + 1) * 128, :])
            xs.append(x_t)
        for c in range(KCH):
            tpb = s_ps_tile([128, 8, 128], "tpb_s")
            for i in range(8):
                nc.tensor.transpose(
                    tpb[:, i, :], xs[i][:, c * 128:(c + 1) * 128], ident)
            dst = xt_tiles[c]
            src = tpb.rearrange("p a b -> p (a b)")
            if c % 2 == 0:
                nc.vector.tensor_copy(dst, src)
            else:
                nc.scalar.copy(dst, src)

    emit_xt_half(0)

    for c in range(KCH):
        wqs = pool.tile([128, 2 * DIM], F32, tag="wqstage", bufs=1,
                        name="wqs")
        nc.sync.dma_start(
            out=wqs, in_=wqkv_d[c * 128:(c + 1) * 128, 0:2 * DIM])
        nc.vector.tensor_copy(wqk[c], wqs)
        wps = pool.tile([128, DIM], F32, tag="wpstage", bufs=1,
                        name="wps")
        nc.sync.dma_start(out=wps, in_=wp_d[c * 128:(c + 1) * 128, :])
        nc.gpsimd.tensor_copy(wp[c], wps)

    # ---------------- V = X_c @ Wv ----------------
    for t_i in range(nkc):
        v_ps = (misc_ps([128, DIM], "v_ps") if t_i % 2 == 0
                else s_ps_tile([128, DIM], "v_ps_s"))
        for c in range(KCH):
            nc.tensor.matmul(v_ps[:, 0:512],
                             xct[c][:, t_i * 128:(t_i + 1) * 128],
                             wv[c][:, 0:512],
                             start=(c == 0), stop=(c == KCH - 1))
            nc.tensor.matmul(v_ps[:, 512:DIM],
                             xct[c][:, t_i * 128:(t_i + 1) * 128],
                             wv[c][:, 512:DIM],
                             start=(c == 0), stop=(c == KCH - 1))
        if t_i % 2 == 0:
            nc.scalar.copy(v_nat[t_i], v_ps)
        else:
            nc.vector.tensor_copy(v_nat[t_i], v_ps)

    # ---------------- Q^T (per quarter) and K^T ----------------
    qt_q = [[None] * KCH for _ in range(QQ)]

    def emit_qt(m, qq):
        xt_tiles = xt_half[qq // 2]
        xoff = (qq % 2) * QW
        qtile = pool.tile([128, QW], BF16, tag=f"qt{m}", bufs=2,
                          name=f"qt{m}q")
        qt_q[qq][m] = qtile
        mm_ps = misc_ps([128, QW], "qk_ps")
        for c in range(KCH):
            nc.tensor.matmul(
                mm_ps, wqk[c][:, m * 128:(m + 1) * 128],
                xt_tiles[c][:, xoff:xoff + QW],
                start=(c == 0), stop=(c == KCH - 1))
        if m % 2 == 0:
            nc.scalar.copy(qtile, mm_ps)
        else:
            nc.vector.tensor_copy(qtile, mm_ps)

    def emit_kt(m, use_s=False):
        wcol = DIM + m * 128
        off = 0
        while off < NK:
            w = min(512, NK - off)
            mm_ps = (s_ps_tile([128, QW], "qk_ps_s") if use_s
                     else misc_ps([128, QW], "qk_ps"))
            for c in range(KCH):
                nc.tensor.matmul(
                    mm_ps[:, 0:w], wqk[c][:, wcol:wcol + 128],
                    xct[c][:, off:off + w],
                    start=(c == 0), stop=(c == KCH - 1))
            if (off // 512) % 2 == 0:
                nc.scalar.copy(kt[m][:, off:off + w], mm_ps[:, 0:w])
            else:
                nc.vector.tensor_copy(kt[m][:, off:off + w],
                                      mm_ps[:, 0:w])
            off += w

    # ---------------- attention for one (qq, hp) ----------------
    def emit_attn(qq, hp, rc_sb, close_prev):
        kt_c = kt[hp]
        qt_c = qt_q[qq][hp]
        o_ps = ps.tile([128, QW], F32, tag="o", bufs=2, name="o_ps")
        s_tiles = {}

        def emit_s(j):
            sp = s_ps_tile([128, 2, QW], "s_pair")
            for a in range(2):
                r0 = a * 64
                nc.tensor.matmul(
                    sp[:, a, :],
                    kt_c[r0:r0 + 64, j * 128:(j + 1) * 128],
                    qt_c[r0:r0 + 64, :],
                    start=True, stop=True)
            s_tiles[j] = sp

        emit_s(0)
        if close_prev is not None:
            close_prev()
        if nkc > 1:
            emit_s(1)
        rs = pool.tile([128, 2, QW], BF16, tag="rs", bufs=3, name="rs")
        rv = rs.rearrange("p a b -> p (a b)")
        for j in range(nkc):
            s_pair = s_tiles.pop(j)
            p_pair = pool.tile([128, 2, QW], BF16, tag="p", bufs=5,
                               name="p_pair")
            s_view = s_pair.rearrange("p a b -> p (a b)")
            p_view = p_pair.rearrange("p a b -> p (a b)")
            if dve_exp(j, hp):
                nc.vector.tensor_scalar(
                    p_view.bitcast(I16), s_view,
                    ss_t[:, j:j + 1], sb_t[:, j:j + 1], MUL, ADD)
            else:
                nc.scalar.activation(p_view, s_view, Exp,
                                     bias=kb_t[:, j:j + 1], scale=SCALE)
            for a in range(2):
                h = 2 * hp + a
                nc.tensor.matmul(
                    o_ps[a * 64:(a + 1) * 64, :],
                    v_nat[j][:, h * D:(h + 1) * D],
                    p_pair[:, a, :],
                    start=(j == 0), stop=(j == nkc - 1),
                    tile_position=(0, a * 64),
                    skip_group_check=True)
            if j + 2 < nkc:
                emit_s(j + 2)
            reng = nc.gpsimd if gps_rs(j, hp) else nc.vector
            if j == 0:
                reng.tensor_copy(rv, p_view)
            else:
                reng.tensor_add(rv, rv, p_view)
        # evacuate unnormalized O~ (bf16); normalize later
        oe = pool.tile([128, QW], BF16, tag="oe", bufs=7, name="oe")
        if hp % 2 == 0:
            nc.vector.tensor_copy(oe, o_ps)
        else:
            nc.scalar.copy(oe, o_ps)

        def close():
            dn_hp = misc_ps([2, QW], "dn_hp")
            for a in range(2):
                nc.tensor.matmul(
                    dn_hp, er[:, 2 * hp + a, 2 * hp:2 * hp + 2],
                    rs[:, a, :],
                    start=(a == 0), stop=(a == 1))
            nc.vector.reciprocal_approx_fast(
                out=rc_sb[:, hp, :], in_=dn_hp)

        return oe, close

    # ---------------- per-quarter tail ----------------
    ot = [[None] * QQ for _ in range(KCH)]

    def emit_qq_tail(qq, oe_l, rc_sb):
        rc_dram = dr.tile([2, KCH, QW], F32, tag="rc_dram", bufs=2,
                          name="rc_dram")
        nc.sync.dma_start(out=rc_dram, in_=rc_sb)
        for hp in range(KCH):
            rc_bc = pool.tile([128, QW], F32, tag="rc_bc", bufs=3,
                              name="rc_bc")
            for a in range(2):
                row = rc_dram[a:a + 1, hp:hp + 1, :]
                bc_ap = bass.AP(
                    tensor=row.tensor, offset=row.offset,
                    ap=[[0, 64]] + [list(p) for p in row.ap[1:]])
                nc.sync.dma_start(out=rc_bc[a * 64:(a + 1) * 64, :],
                                  in_=bc_ap)
            ott = pool.tile([128, QW], BF16, tag="ot", bufs=7, name="ott")
            nc.vector.tensor_mul(ott, oe_l[hp], rc_bc)
            ot[hp][qq] = ott
        tq = NCH // QQ
        for t_i in range(qq * tq, (qq + 1) * tq):
            tl = (t_i % tq) * 128
            pr_ps = misc_ps([128, 2, 512], "pr_ps")
            for c in range(KCH):
                nc.tensor.matmul(
                    pr_ps[:, 0, :], ot[c][qq][:, tl:tl + 128],
                    wp[c][:, 0:512],
                    start=(c == 0), stop=(c == KCH - 1))
                nc.tensor.matmul(
                    pr_ps[:, 1, 0:256], ot[c][qq][:, tl:tl + 128],
                    wp[c][:, 512:DIM],
                    start=(c == 0), stop=(c == KCH - 1))
            out_t = pool.tile([128, DIM], F32, tag="out_t", bufs=2,
                              name="out_t")
            if t_i % 2 == 0:
                nc.scalar.copy(
                    out_t, pr_ps.rearrange("p a b -> p (a b)")[:, 0:DIM])
            else:
                nc.vector.tensor_copy(
                    out_t, pr_ps.rearrange("p a b -> p (a b)")[:, 0:DIM])
            nc.sync.dma_start(
                out=o_d[t_i * 128:(t_i + 1) * 128, :], in_=out_t)

    # ---------------- wavefront emission ----------------
    emit_kt(0, use_s=True)
    emit_qt(0, 0)
    for qq in range(QQ):
        rc_sb = pool.tile([2, KCH, QW], F32, tag="rc_sb", bufs=1,
                          name="rc_sb")
        oe_l = []
        close = None
        for hp in range(KCH):
            if qq == 0 and hp + 1 < KCH:
                emit_kt(hp + 1)
                emit_qt(hp + 1, 0)
            oe, close = emit_attn(qq, hp, rc_sb, close)
            oe_l.append(oe)
        close()
        if qq == 1:
            emit_xt_half(1)
        if qq + 1 < QQ:
            for m in range(KCH):
                emit_qt(m, qq + 1)
        emit_qq_tail(qq, oe_l, rc_sb)

    pool.release()
    ps.release()
    dr.release()


_CACHE = {}


def _get_compiled(nkc):
    if nkc in _CACHE:
        return _CACHE[nkc]
    NK = nkc * 128
    nc = bacc.Bacc("TRN2", target_bir_lowering=False, debug=False,
                   num_devices=B)
    x_d = nc.dram_tensor("x", [N, DIM], F32, kind="ExternalInput").ap()
    ki_d = nc.dram_tensor("kidx", [NK], I32, kind="ExternalInput").ap()
    kb_d = nc.dram_tensor("kbias", [NK], F32, kind="ExternalInput").ap()
    ss_d = nc.dram_tensor("sscale", [NK], F32, kind="ExternalInput").ap()
    sb_d = nc.dram_tensor("sbias", [NK], F32, kind="ExternalInput").ap()
    wqkv_d = nc.dram_tensor("w_qkv", [DIM, 3 * DIM], F32,
                            kind="ExternalInput").ap()
    wp_d = nc.dram_tensor("w_proj", [DIM, DIM], F32,
                          kind="ExternalInput").ap()
    o_d = nc.dram_tensor("out", [N, DIM], F32, kind="ExternalOutput").ap()
    with tile.TileContext(nc) as tc:
        _build(nc, tc, (x_d, ki_d, kb_d, ss_d, sb_d, wqkv_d, wp_d, o_d),
               nkc)
    nc.compile()
    _CACHE[nkc] = nc
    return nc


def prep_run(x, mask, w_qkv, w_proj, b_proj):
    x = np.ascontiguousarray(np.asarray(x, dtype=np.float32))
    mask = np.ascontiguousarray(np.asarray(mask, dtype=np.int32))
    w_qkv = np.ascontiguousarray(np.asarray(w_qkv, dtype=np.float32))
    w_proj = np.ascontiguousarray(np.asarray(w_proj, dtype=np.float32))

    idxs = [np.flatnonzero(mask[b]).astype(np.int32) for b in range(B)]
    max_valid = max(len(i) for i in idxs)
    nkc = min(NCH, max(1, -(-max_valid // 128)))
    NK = nkc * 128
    kidx = np.zeros((B, NK), dtype=np.int32)
    kbias = np.full((B, NK), -1.0e30, dtype=np.float32)
    sscale = np.zeros((B, NK), dtype=np.float32)
    sbias = np.zeros((B, NK), dtype=np.float32)
    for b in range(B):
        n = len(idxs[b])
        kidx[b, :n] = idxs[b]
        kbias[b, :n] = 0.0
        sscale[b, :n] = SCHR_SCALE
        sbias[b, :n] = SCHR_BIAS

    nc = _get_compiled(nkc)
    in_maps = [
        {"x": x[b], "kidx": kidx[b], "kbias": kbias[b],
         "sscale": sscale[b], "sbias": sbias[b],
         "w_qkv": w_qkv, "w_proj": w_proj}
        for b in range(B)
    ]
    return nc, in_maps


def kernel(x, mask, w_qkv, w_proj, b_proj):
    nc, in_maps = prep_run(x, mask, w_qkv, w_proj, b_proj)
    b_proj = np.asarray(b_proj, dtype=np.float32)
    last_err = None
    for _ in range(3):
        try:
            res = run_bass_kernel_spmd(nc, in_maps, list(range(B))).results
            out = np.stack([res[b]["out"] for b in range(B)], axis=0)
            return out + b_proj
        except Exception as e:  # transient device hiccup: retry
            last_err = e
    raise last_err


# revision 14
# speedup vs baseline: 1.7578x; 1.0314x over previous
"""Trainium2 Bass kernel: masked multi-head self-attention block.

out = softmax_mask((x @ Wq) (x @ Wk)^T / sqrt(d)) (x @ Wv) @ Wp + b

Sharding: data-parallel over batch B=8 across the 8 NeuronCores (one
batch row per core); weights replicated; no collectives.

Key compaction: only valid key rows (mask==1) participate; indices are
computed on the host, rows gathered on-device via indirect DMA, padded
to a multiple of 128 (pad slots produce exp()==0).

v3 design (after the latency-bound v2 at 826us):
- Query dim processed in 512-wide quarters; S tiles are [128,2a,512]
  f32 (2 PSUM banks) holding BOTH heads of a pair, so exp runs as one
  [128,1024] instruction per key chunk and the dedicated "s" PSUM tag
  (2 slots) gives 2-key-chunk pipeline depth.  A separate "misc" tag
  serves QKV/proj/transpose/denominator matmuls so they never starve
  the S rotation (the v2 mistake), and "o" (2x 1-bank slots) holds the
  PV accumulators.
- exp engine split: ScalarE for 6 of 9 key chunks, DVE for chunks
  {3,5,7} via the one-op Schraudolph bit-trick (i16 = floor(S*(A/8)+B)
  is the bf16 exp bit pattern); per-partition scale/bias double as the
  pad mask.
- softmax row-sums: bf16 partial adds split DVE/GpSimd by key chunk;
  cross-partition reduction via two accumulating e_r matmuls per
  (quarter, head-pair) into a rotating [2,512] PSUM tile, evacuated to
  a per-quarter [12,512] table; one reciprocal + DRAM broadcast per
  quarter; O~ evacuated unnormalized (bf16) and scaled bf16*bf16.
- x transposes on PE in f32 with 8-wide batched evacuations; the xt
  stream borrows the idle "s" slots during lead-in while xct uses
  "misc", so the two streams run concurrently.
- b_proj is added on the host.
"""
import numpy as np

import concourse.bass as bass
import concourse.tile as tile
from concourse import bacc, mybir
from concourse.bass_utils import run_bass_kernel_spmd
from concourse.masks import make_identity

F32 = mybir.dt.float32
BF16 = mybir.dt.bfloat16
I16 = mybir.dt.int16
I32 = mybir.dt.int32

B, N, DIM = 8, 2048, 768
H, D = 12, 64
SCALE = D ** -0.5
NCH = N // 128        # 16 token chunks
KCH = DIM // 128      # 6 feature chunks
QQ = 4                # query quarters
QW = N // QQ          # 512 queries per quarter
Exp = mybir.ActivationFunctionType.Exp
MUL = mybir.AluOpType.mult
ADD = mybir.AluOpType.add

# Schraudolph constants (floor conversion semantics on DVE)
A16 = 128.0 / float(np.log(2.0))
SCHR_C = 5.1
SCHR_SCALE = A16 * SCALE
SCHR_BIAS = 127.0 * 128.0 - SCHR_C

def dve_exp(j, hp):
    return j == 5 or (j == 7 and hp % 2 == 0)


def gps_rs(j, hp):
    return j in (1, 4, 6)


def _build(nc, tc, aps, nkc):
    (x_d, ki_d, kb_d, ss_d, sb_d, wqkv_d, wp_d, o_d) = aps
    NK = nkc * 128

    pool = tc.alloc_tile_pool(name="sb", bufs=1)
    ps = tc.alloc_tile_pool(name="ps", bufs=1, space="PSUM")
    dr = tc.alloc_tile_pool(name="dr", bufs=1, space="DRAM")

    def s_ps_tile(shape, name):
        return ps.tile(shape, F32, tag="s", bufs=3, name=name)

    def misc_ps(shape, name):
        return ps.tile(shape, F32, tag="misc", bufs=1, name=name)

    # ---------------- constants ----------------
    ident = pool.tile([128, 128], F32, tag="ident")
    make_identity(nc, ident)
    er = pool.tile([128, 12, 12], BF16, tag="er")
    nc.vector.memset(er, 0.0)
    for r in range(12):
        nc.vector.memset(er[:, r, r:r + 1], 1.0)
    kb_t = pool.tile([128, nkc], F32, tag="kb")
    nc.sync.dma_start(out=kb_t, in_=kb_d.rearrange("(j p) -> p j", p=128))
    ss_t = pool.tile([128, nkc], F32, tag="ss")
    nc.sync.dma_start(out=ss_t, in_=ss_d.rearrange("(j p) -> p j", p=128))
    sb_t = pool.tile([128, nkc], F32, tag="sbv")
    nc.sync.dma_start(out=sb_t, in_=sb_d.rearrange("(j p) -> p j", p=128))
    ki_t = pool.tile([128, nkc], I32, tag="ki")
    nc.sync.dma_start(out=ki_t, in_=ki_d.rearrange("(j p) -> p j", p=128))

    # ---------------- persistent tiles ----------------
    xct = [pool.tile([128, NK], BF16, tag=f"xct{c}", name=f"xct{c}")
           for c in range(KCH)]
    kt = [pool.tile([128, NK], BF16, tag=f"kt{m}", name=f"kt{m}")
          for m in range(KCH)]
    v_nat = [pool.tile([128, DIM], BF16, tag=f"vn{t}", name=f"vn{t}")
             for t in range(nkc)]
    wqk = [pool.tile([128, 2 * DIM], BF16, tag=f"wqk{c}", name=f"wqk{c}")
           for c in range(KCH)]
    wv = [pool.tile([128, DIM], BF16, tag=f"wv{c}", name=f"wv{c}")
          for c in range(KCH)]
    wp = [pool.tile([128, DIM], BF16, tag=f"wp{c}", name=f"wp{c}")
          for c in range(KCH)]

    # ---------------- gathered X_c^T (all gathers first) ------------
    nb = 0
    while nb < nkc:
        bw = min(4, nkc - nb)
        xg_l = []
        for t_i in range(nb, nb + bw):
            xg = pool.tile([128, DIM], F32, tag="xg", bufs=8, name="xg")
            nc.gpsimd.indirect_dma_start(
                out=xg, out_offset=None, in_=x_d,
                in_offset=bass.IndirectOffsetOnAxis(
                    ap=ki_t[:, t_i:t_i + 1], axis=0))
            xg_l.append(xg)
        for c in range(KCH):
            tpb = misc_ps([128, 4, 128], "tpb")
            for i in range(bw):
                nc.tensor.transpose(
                    tpb[:, i, :], xg_l[i][:, c * 128:(c + 1) * 128],
                    ident)
            dst = xct[c][:, nb * 128:(nb + bw) * 128]
            src = tpb.rearrange("p a b -> p (a b)")[:, 0:bw * 128]
            if c % 2 == 0:
                nc.scalar.copy(dst, src)
            else:
                nc.vector.tensor_copy(dst, src)
        nb += bw

    # ---------------- weights: DMA + cast (V first) ----------------
    for c in range(KCH):
        wvs = pool.tile([128, DIM], F32, tag="wvstage", bufs=1,
                        name="wvs")
        nc.sync.dma_start(
            out=wvs, in_=wqkv_d[c * 128:(c + 1) * 128, 2 * DIM:3 * DIM])
        nc.gpsimd.tensor_copy(wv[c], wvs)

    # ---------------- X^T half tiles (8-batched transposes) ---------
    xt_half = {}

    def emit_xt_half(qh):
        xt_tiles = [pool.tile([128, N // 2], BF16, tag=f"xt{c}", bufs=1,
                              name=f"xt{c}h")
                    for c in range(KCH)]
        xt_half[qh] = xt_tiles
        xs = []
        for t in range(qh * 8, qh * 8 + 8):
            x_t = pool.tile([128, DIM], F32, tag="xg", bufs=8, name="x_t")
            nc.sync.dma_start(out=x_t, in_=x_d[t * 128:(t + 1) * 128, :])
            xs.append(x_t)
        for c in range(KCH):
            tpb = s_ps_tile([128, 8, 128], "tpb_s")
            for i in range(8):
                nc.tensor.transpose(
                    tpb[:, i, :], xs[i][:, c * 128:(c + 1) * 128], ident)
            dst = xt_tiles[c]
            src = tpb.rearrange("p a b -> p (a b)")
            if c % 2 == 0:
                nc.vector.tensor_copy(dst, src)
            else:
                nc.scalar.copy(dst, src)

    emit_xt_half(0)

    for c in range(KCH):
        wqs = pool.tile([128, 2 * DIM], F32, tag="wqstage", bufs=1,
                        name="wqs")
        nc.sync.dma_start(
            out=wqs, in_=wqkv_d[c * 128:(c + 1) * 128, 0:2 * DIM])
        nc.vector.tensor_copy(wqk[c], wqs)
        wps = pool.tile([128, DIM], F32, tag="wpstage", bufs=1,
                        name="wps")
        nc.sync.dma_start(out=wps, in_=wp_d[c * 128:(c + 1) * 128, :])
        nc.gpsimd.tensor_copy(wp[c], wps)

    # ---------------- V = X_c @ Wv ----------------
    for t_i in range(nkc):
        for half, (lo, hi) in enumerate(((0, 512), (512, DIM))):
            v_ps = (misc_ps([128, 512], "v_ps") if (t_i + half) % 2 == 0
                    else s_ps_tile([128, 512], "v_ps_s"))
            for c in range(KCH):
                nc.tensor.matmul(v_ps[:, 0:hi - lo],
                                 xct[c][:, t_i * 128:(t_i + 1) * 128],
                                 wv[c][:, lo:hi],
                                 start=(c == 0), stop=(c == KCH - 1))
            if (t_i + half) % 2 == 0:
                nc.scalar.copy(v_nat[t_i][:, lo:hi], v_ps[:, 0:hi - lo])
            else:
                nc.vector.tensor_copy(v_nat[t_i][:, lo:hi],
                                      v_ps[:, 0:hi - lo])

    # ---------------- Q^T (per quarter) and K^T ----------------
    qt_q = [[None] * KCH for _ in range(QQ)]

    def emit_qt(m, qq):
        xt_tiles = xt_half[qq // 2]
        xoff = (qq % 2) * QW
        qtile = pool.tile([128, QW], BF16, tag=f"qt{m}", bufs=2,
                          name=f"qt{m}q")
        qt_q[qq][m] = qtile
        mm_ps = misc_ps([128, QW], "qk_ps")
        for c in range(KCH):
            nc.tensor.matmul(
                mm_ps, wqk[c][:, m * 128:(m + 1) * 128],
                xt_tiles[c][:, xoff:xoff + QW],
                start=(c == 0), stop=(c == KCH - 1))
        if m % 2 == 0:
            nc.scalar.copy(qtile, mm_ps)
        else:
            nc.vector.tensor_copy(qtile, mm_ps)

    def emit_kt(m, use_s=False):
        wcol = DIM + m * 128
        off = 0
        while off < NK:
            w = min(512, NK - off)
            mm_ps = (s_ps_tile([128, QW], "qk_ps_s") if use_s
                     else misc_ps([128, QW], "qk_ps"))
            for c in range(KCH):
                nc.tensor.matmul(
                    mm_ps[:, 0:w], wqk[c][:, wcol:wcol + 128],
                    xct[c][:, off:off + w],
                    start=(c == 0), stop=(c == KCH - 1))
            if (off // 512) % 2 == 0:
                nc.scalar.copy(kt[m][:, off:off + w], mm_ps[:, 0:w])
            else:
                nc.vector.tensor_copy(kt[m][:, off:off + w],
                                      mm_ps[:, 0:w])
            off += w

    # ---------------- attention for one (qq, hp) ----------------
    def emit_attn(qq, hp, rc_sb, close_prev):
        kt_c = kt[hp]
        qt_c = qt_q[qq][hp]
        o_ps = ps.tile([128, QW], F32, tag="o", bufs=1, name="o_ps")
        s_tiles = {}

        def emit_s(j):
            sp = s_ps_tile([128, 2, QW], "s_pair")
            for a in range(2):
                r0 = a * 64
                nc.tensor.matmul(
                    sp[:, a, :],
                    kt_c[r0:r0 + 64, j * 128:(j + 1) * 128],
                    qt_c[r0:r0 + 64, :],
                    start=True, stop=True)
            s_tiles[j] = sp

        emit_s(0)
        if close_prev is not None:
            close_prev()
        if nkc > 1:
            emit_s(1)
        rs = pool.tile([128, 2, QW], BF16, tag="rs", bufs=3, name="rs")
        rv = rs.rearrange("p a b -> p (a b)")
        for j in range(nkc):
            s_pair = s_tiles.pop(j)
            p_pair = pool.tile([128, 2, QW], BF16, tag="p", bufs=5,
                               name="p_pair")
            s_view = s_pair.rearrange("p a b -> p (a b)")
            p_view = p_pair.rearrange("p a b -> p (a b)")
            if dve_exp(j, hp):
                nc.vector.tensor_scalar(
                    p_view.bitcast(I16), s_view,
                    ss_t[:, j:j + 1], sb_t[:, j:j + 1], MUL, ADD)
            else:
                nc.scalar.activation(p_view, s_view, Exp,
                                     bias=kb_t[:, j:j + 1], scale=SCALE)
            for a in range(2):
                h = 2 * hp + a
                nc.tensor.matmul(
                    o_ps[a * 64:(a + 1) * 64, :],
                    v_nat[j][:, h * D:(h + 1) * D],
                    p_pair[:, a, :],
                    start=(j == 0), stop=(j == nkc - 1),
                    tile_position=(0, a * 64),
                    skip_group_check=True)
            if j + 2 < nkc:
                emit_s(j + 2)
            reng = nc.gpsimd if gps_rs(j, hp) else nc.vector
            if j == 0:
                reng.tensor_copy(rv, p_view)
            else:
                reng.tensor_add(rv, rv, p_view)
        # evacuate unnormalized O~ (bf16); normalize later
        oe = pool.tile([128, QW], BF16, tag="oe", bufs=7, name="oe")
        if hp % 2 == 0:
            nc.vector.tensor_copy(oe, o_ps)
        else:
            nc.scalar.copy(oe, o_ps)

        def close():
            dn_hp = misc_ps([2, QW], "dn_hp")
            for a in range(2):
                nc.tensor.matmul(
                    dn_hp, er[:, 2 * hp + a, 2 * hp:2 * hp + 2],
                    rs[:, a, :],
                    start=(a == 0), stop=(a == 1))
            nc.vector.reciprocal_approx_fast(
                out=rc_sb[:, hp, :], in_=dn_hp)

        return oe, close

    # ---------------- per-quarter tail ----------------
    ot = [[None] * QQ for _ in range(KCH)]

    def emit_qq_tail(qq, oe_l, rc_sb):
        rc_dram = dr.tile([2, KCH, QW], F32, tag="rc_dram", bufs=2,
                          name="rc_dram")
        nc.sync.dma_start(out=rc_dram, in_=rc_sb)
        for hp in range(KCH):
            rc_bc = pool.tile([128, QW], F32, tag="rc_bc", bufs=3,
                              name="rc_bc")
            for a in range(2):
                row = rc_dram[a:a + 1, hp:hp + 1, :]
                bc_ap = bass.AP(
                    tensor=row.tensor, offset=row.offset,
                    ap=[[0, 64]] + [list(p) for p in row.ap[1:]])
                nc.sync.dma_start(out=rc_bc[a * 64:(a + 1) * 64, :],
                                  in_=bc_ap)
            ott = pool.tile([128, QW], BF16, tag="ot", bufs=7, name="ott")
            nc.vector.tensor_mul(ott, oe_l[hp], rc_bc)
            ot[hp][qq] = ott
        tq = NCH // QQ
        for t_i in range(qq * tq, (qq + 1) * tq):
            tl = (t_i % tq) * 128
            out_t = pool.tile([128, DIM], F32, tag="out_t", bufs=2,
                              name="out_t")
            for half, (lo, hi) in enumerate(((0, 512), (512, DIM))):
                pr_ps = misc_ps([128, 512], "pr_ps")
                for c in range(KCH):
                    nc.tensor.matmul(
                        pr_ps[:, 0:hi - lo], ot[c][qq][:, tl:tl + 128],
                        wp[c][:, lo:hi],
                        start=(c == 0), stop=(c == KCH - 1))
                if (t_i + half) % 2 == 0:
                    nc.scalar.copy(out_t[:, lo:hi], pr_ps[:, 0:hi - lo])
                else:
                    nc.vector.tensor_copy(out_t[:, lo:hi],
                                          pr_ps[:, 0:hi - lo])
            nc.sync.dma_start(
                out=o_d[t_i * 128:(t_i + 1) * 128, :], in_=out_t)

    # ---------------- wavefront emission ----------------
    emit_kt(0, use_s=True)
    emit_qt(0, 0)
    for qq in range(QQ):
        rc_sb = pool.tile([2, KCH, QW], F32, tag="rc_sb", bufs=1,
                          name="rc_sb")
        oe_l = []
        close = None
        for hp in range(KCH):
            if qq == 0 and hp + 1 < KCH:
                emit_kt(hp + 1)
                emit_qt(hp + 1, 0)
            oe, close = emit_attn(qq, hp, rc_sb, close)
            oe_l.append(oe)
        close()
        if qq == 1:
            emit_xt_half(1)
        if qq + 1 < QQ:
            for m in range(KCH):
                emit_qt(m, qq + 1)
        emit_qq_tail(qq, oe_l, rc_sb)

    pool.release()
    ps.release()
    dr.release()


_CACHE = {}


def _get_compiled(nkc):
    if nkc in _CACHE:
        return _CACHE[nkc]
    NK = nkc * 128
    nc = bacc.Bacc("TRN2", target_bir_lowering=False, debug=False,
                   num_devices=B)
    x_d = nc.dram_tensor("x", [N, DIM], F32, kind="ExternalInput").ap()
    ki_d = nc.dram_tensor("kidx", [NK], I32, kind="ExternalInput").ap()
    kb_d = nc.dram_tensor("kbias", [NK], F32, kind="ExternalInput").ap()
    ss_d = nc.dram_tensor("sscale", [NK], F32, kind="ExternalInput").ap()
    sb_d = nc.dram_tensor("sbias", [NK], F32, kind="ExternalInput").ap()
    wqkv_d = nc.dram_tensor("w_qkv", [DIM, 3 * DIM], F32,
                            kind="ExternalInput").ap()
    wp_d = nc.dram_tensor("w_proj", [DIM, DIM], F32,
                          kind="ExternalInput").ap()
    o_d = nc.dram_tensor("out", [N, DIM], F32, kind="ExternalOutput").ap()
    with tile.TileContext(nc) as tc:
        _build(nc, tc, (x_d, ki_d, kb_d, ss_d, sb_d, wqkv_d, wp_d, o_d),
               nkc)
    nc.compile()
    _CACHE[nkc] = nc
    return nc


def prep_run(x, mask, w_qkv, w_proj, b_proj):
    x = np.ascontiguousarray(np.asarray(x, dtype=np.float32))
    mask = np.ascontiguousarray(np.asarray(mask, dtype=np.int32))
    w_qkv = np.ascontiguousarray(np.asarray(w_qkv, dtype=np.float32))
    w_proj = np.ascontiguousarray(np.asarray(w_proj, dtype=np.float32))

    idxs = [np.flatnonzero(mask[b]).astype(np.int32) for b in range(B)]
    max_valid = max(len(i) for i in idxs)
    nkc = min(NCH, max(1, -(-max_valid // 128)))
    NK = nkc * 128
    kidx = np.zeros((B, NK), dtype=np.int32)
    kbias = np.full((B, NK), -1.0e30, dtype=np.float32)
    sscale = np.zeros((B, NK), dtype=np.float32)
    sbias = np.zeros((B, NK), dtype=np.float32)
    for b in range(B):
        n = len(idxs[b])
        kidx[b, :n] = idxs[b]
        kbias[b, :n] = 0.0
        sscale[b, :n] = SCHR_SCALE
        sbias[b, :n] = SCHR_BIAS

    nc = _get_compiled(nkc)
    in_maps = [
        {"x": x[b], "kidx": kidx[b], "kbias": kbias[b],
         "sscale": sscale[b], "sbias": sbias[b],
         "w_qkv": w_qkv, "w_proj": w_proj}
        for b in range(B)
    ]
    return nc, in_maps


def kernel(x, mask, w_qkv, w_proj, b_proj):
    nc, in_maps = prep_run(x, mask, w_qkv, w_proj, b_proj)
    b_proj = np.asarray(b_proj, dtype=np.float32)
    last_err = None
    for _ in range(3):
        try:
            res = run_bass_kernel_spmd(nc, in_maps, list(range(B))).results
            out = np.stack([res[b]["out"] for b in range(B)], axis=0)
            return out + b_proj
        except Exception as e:  # transient device hiccup: retry
            last_err = e
    raise last_err


# revision 15
# speedup vs baseline: 1.7783x; 1.0117x over previous
"""Trainium2 Bass kernel: masked multi-head self-attention block.

out = softmax_mask((x @ Wq) (x @ Wk)^T / sqrt(d)) (x @ Wv) @ Wp + b

Sharding: data-parallel over batch B=8 across the 8 NeuronCores (one
batch row per core); weights replicated; no collectives.

Key compaction: only valid key rows (mask==1) participate; indices are
computed on the host, rows gathered on-device via indirect DMA, padded
to a multiple of 128 (pad slots produce exp()==0).

v3 design (after the latency-bound v2 at 826us):
- Query dim processed in 512-wide quarters; S tiles are [128,2a,512]
  f32 (2 PSUM banks) holding BOTH heads of a pair, so exp runs as one
  [128,1024] instruction per key chunk and the dedicated "s" PSUM tag
  (2 slots) gives 2-key-chunk pipeline depth.  A separate "misc" tag
  serves QKV/proj/transpose/denominator matmuls so they never starve
  the S rotation (the v2 mistake), and "o" (2x 1-bank slots) holds the
  PV accumulators.
- exp engine split: ScalarE for 6 of 9 key chunks, DVE for chunks
  {3,5,7} via the one-op Schraudolph bit-trick (i16 = floor(S*(A/8)+B)
  is the bf16 exp bit pattern); per-partition scale/bias double as the
  pad mask.
- softmax row-sums: bf16 partial adds split DVE/GpSimd by key chunk;
  cross-partition reduction via two accumulating e_r matmuls per
  (quarter, head-pair) into a rotating [2,512] PSUM tile, evacuated to
  a per-quarter [12,512] table; one reciprocal + DRAM broadcast per
  quarter; O~ evacuated unnormalized (bf16) and scaled bf16*bf16.
- x transposes on PE in f32 with 8-wide batched evacuations; the xt
  stream borrows the idle "s" slots during lead-in while xct uses
  "misc", so the two streams run concurrently.
- b_proj is added on the host.
"""
import numpy as np

import concourse.bass as bass
import concourse.tile as tile
from concourse import bacc, mybir
from concourse.bass_utils import run_bass_kernel_spmd
from concourse.masks import make_identity

F32 = mybir.dt.float32
BF16 = mybir.dt.bfloat16
I16 = mybir.dt.int16
I32 = mybir.dt.int32

B, N, DIM = 8, 2048, 768
H, D = 12, 64
SCALE = D ** -0.5
NCH = N // 128        # 16 token chunks
KCH = DIM // 128      # 6 feature chunks
QQ = 4                # query quarters
QW = N // QQ          # 512 queries per quarter
Exp = mybir.ActivationFunctionType.Exp
MUL = mybir.AluOpType.mult
ADD = mybir.AluOpType.add

# Schraudolph constants (floor conversion semantics on DVE)
A16 = 128.0 / float(np.log(2.0))
SCHR_C = 5.1
SCHR_SCALE = A16 * SCALE
SCHR_BIAS = 127.0 * 128.0 - SCHR_C

def dve_exp(j, hp):
    return j == 5 or (j == 7 and hp % 2 == 0)


def gps_rs(j, hp):
    return j in (1, 4, 6)


def _build(nc, tc, aps, nkc):
    (x_d, ki_d, kb_d, ss_d, sb_d, wqkv_d, wp_d, o_d) = aps
    NK = nkc * 128

    pool = tc.alloc_tile_pool(name="sb", bufs=1)
    ps = tc.alloc_tile_pool(name="ps", bufs=1, space="PSUM")
    dr = tc.alloc_tile_pool(name="dr", bufs=1, space="DRAM")

    def s_ps_tile(shape, name):
        return ps.tile(shape, F32, tag="s", bufs=3, name=name)

    def misc_ps(shape, name):
        return ps.tile(shape, F32, tag="misc", bufs=1, name=name)

    # ---------------- constants ----------------
    ident = pool.tile([128, 128], F32, tag="ident")
    make_identity(nc, ident)
    er = pool.tile([128, 12, 12], BF16, tag="er")
    nc.vector.memset(er, 0.0)
    for r in range(12):
        nc.vector.memset(er[:, r, r:r + 1], 1.0)
    kb_t = pool.tile([128, nkc], F32, tag="kb")
    nc.sync.dma_start(out=kb_t, in_=kb_d.rearrange("(j p) -> p j", p=128))
    ss_t = pool.tile([128, nkc], F32, tag="ss")
    nc.sync.dma_start(out=ss_t, in_=ss_d.rearrange("(j p) -> p j", p=128))
    sb_t = pool.tile([128, nkc], F32, tag="sbv")
    nc.sync.dma_start(out=sb_t, in_=sb_d.rearrange("(j p) -> p j", p=128))
    ki_t = pool.tile([128, nkc], I32, tag="ki")
    nc.sync.dma_start(out=ki_t, in_=ki_d.rearrange("(j p) -> p j", p=128))

    # ---------------- persistent tiles ----------------
    xct = [pool.tile([128, NK], BF16, tag=f"xct{c}", name=f"xct{c}")
           for c in range(KCH)]
    kt = [pool.tile([128, NK], BF16, tag=f"kt{m}", name=f"kt{m}")
          for m in range(KCH)]
    v_nat = [pool.tile([128, DIM], BF16, tag=f"vn{t}", name=f"vn{t}")
             for t in range(nkc)]
    wqk = [pool.tile([128, 2 * DIM], BF16, tag=f"wqk{c}", name=f"wqk{c}")
           for c in range(KCH)]
    wv = [pool.tile([128, DIM], BF16, tag=f"wv{c}", name=f"wv{c}")
          for c in range(KCH)]
    wp = [pool.tile([128, DIM], BF16, tag=f"wp{c}", name=f"wp{c}")
          for c in range(KCH)]

    # ---------------- gathered X_c^T (all gathers first) ------------
    nb = 0
    while nb < nkc:
        bw = min(4, nkc - nb)
        xg_l = []
        for t_i in range(nb, nb + bw):
            xg = pool.tile([128, DIM], F32, tag="xg", bufs=8, name="xg")
            nc.gpsimd.indirect_dma_start(
                out=xg, out_offset=None, in_=x_d,
                in_offset=bass.IndirectOffsetOnAxis(
                    ap=ki_t[:, t_i:t_i + 1], axis=0))
            xg_l.append(xg)
        for c in range(KCH):
            tpb = misc_ps([128, 4, 128], "tpb")
            for i in range(bw):
                nc.tensor.transpose(
                    tpb[:, i, :], xg_l[i][:, c * 128:(c + 1) * 128],
                    ident)
            dst = xct[c][:, nb * 128:(nb + bw) * 128]
            src = tpb.rearrange("p a b -> p (a b)")[:, 0:bw * 128]
            if c % 2 == 0:
                nc.scalar.copy(dst, src)
            else:
                nc.vector.tensor_copy(dst, src)
        nb += bw

    # ---------------- weights: DMA + cast (V first) ----------------
    for c in range(KCH):
        wvs = pool.tile([128, DIM], F32, tag="wvstage", bufs=1,
                        name="wvs")
        nc.sync.dma_start(
            out=wvs, in_=wqkv_d[c * 128:(c + 1) * 128, 2 * DIM:3 * DIM])
        nc.gpsimd.tensor_copy(wv[c], wvs)

    # ---------------- X^T half tiles (8-batched transposes) ---------
    xt_half = {}

    def emit_xt_half(qh):
        xt_tiles = [pool.tile([128, N // 2], BF16, tag=f"xt{c}", bufs=1,
                              name=f"xt{c}h")
                    for c in range(KCH)]
        xt_half[qh] = xt_tiles
        xs = []
        for t in range(qh * 8, qh * 8 + 8):
            x_t = pool.tile([128, DIM], F32, tag="xg", bufs=8, name="x_t")
            nc.sync.dma_start(out=x_t, in_=x_d[t * 128:(t + 1) * 128, :])
            xs.append(x_t)
        for c in range(KCH):
            tpb = s_ps_tile([128, 8, 128], "tpb_s")
            for i in range(8):
                nc.tensor.transpose(
                    tpb[:, i, :], xs[i][:, c * 128:(c + 1) * 128], ident)
            dst = xt_tiles[c]
            src = tpb.rearrange("p a b -> p (a b)")
            if c % 2 == 0:
                nc.vector.tensor_copy(dst, src)
            else:
                nc.scalar.copy(dst, src)

    emit_xt_half(0)

    for c in range(KCH):
        wqs = pool.tile([128, 2 * DIM], F32, tag="wqstage", bufs=1,
                        name="wqs")
        nc.sync.dma_start(
            out=wqs, in_=wqkv_d[c * 128:(c + 1) * 128, 0:2 * DIM])
        nc.vector.tensor_copy(wqk[c], wqs)
        wps = pool.tile([128, DIM], F32, tag="wpstage", bufs=1,
                        name="wps")
        nc.sync.dma_start(out=wps, in_=wp_d[c * 128:(c + 1) * 128, :])
        nc.gpsimd.tensor_copy(wp[c], wps)

    # ---------------- V = X_c @ Wv ----------------
    for t_i in range(nkc):
        for half, (lo, hi) in enumerate(((0, 512), (512, DIM))):
            v_ps = (misc_ps([128, 512], "v_ps") if (t_i + half) % 2 == 0
                    else s_ps_tile([128, 512], "v_ps_s"))
            for c in range(KCH):
                nc.tensor.matmul(v_ps[:, 0:hi - lo],
                                 xct[c][:, t_i * 128:(t_i + 1) * 128],
                                 wv[c][:, lo:hi],
                                 start=(c == 0), stop=(c == KCH - 1))
            if (t_i + half) % 2 == 0:
                nc.scalar.copy(v_nat[t_i][:, lo:hi], v_ps[:, 0:hi - lo])
            else:
                nc.vector.tensor_copy(v_nat[t_i][:, lo:hi],
                                      v_ps[:, 0:hi - lo])

    # ---------------- Q^T (per quarter) and K^T ----------------
    qt_q = [[None] * KCH for _ in range(QQ)]

    def emit_qt(m, qq):
        xt_tiles = xt_half[qq // 2]
        xoff = (qq % 2) * QW
        qtile = pool.tile([128, QW], BF16, tag=f"qt{m}", bufs=2,
                          name=f"qt{m}q")
        qt_q[qq][m] = qtile
        mm_ps = misc_ps([128, QW], "qk_ps")
        for c in range(KCH):
            nc.tensor.matmul(
                mm_ps, wqk[c][:, m * 128:(m + 1) * 128],
                xt_tiles[c][:, xoff:xoff + QW],
                start=(c == 0), stop=(c == KCH - 1))
        if m % 2 == 0:
            nc.scalar.copy(qtile, mm_ps)
        else:
            nc.vector.tensor_copy(qtile, mm_ps)

    def emit_kt(m, use_s=False):
        wcol = DIM + m * 128
        off = 0
        while off < NK:
            w = min(512, NK - off)
            mm_ps = (s_ps_tile([128, QW], "qk_ps_s") if use_s
                     else misc_ps([128, QW], "qk_ps"))
            for c in range(KCH):
                nc.tensor.matmul(
                    mm_ps[:, 0:w], wqk[c][:, wcol:wcol + 128],
                    xct[c][:, off:off + w],
                    start=(c == 0), stop=(c == KCH - 1))
            if (off // 512) % 2 == 0:
                nc.scalar.copy(kt[m][:, off:off + w], mm_ps[:, 0:w])
            else:
                nc.vector.tensor_copy(kt[m][:, off:off + w],
                                      mm_ps[:, 0:w])
            off += w

    # ---------------- attention for one (qq, hp) ----------------
    def emit_attn(qq, hp, rc_sb, close_prev):
        kt_c = kt[hp]
        qt_c = qt_q[qq][hp]
        o_ps = ps.tile([128, QW], F32, tag="o", bufs=1, name="o_ps")
        s_tiles = {}

        def emit_s(j):
            sp = s_ps_tile([128, 2, QW], "s_pair")
            for a in range(2):
                r0 = a * 64
                nc.tensor.matmul(
                    sp[:, a, :],
                    kt_c[r0:r0 + 64, j * 128:(j + 1) * 128],
                    qt_c[r0:r0 + 64, :],
                    start=True, stop=True)
            s_tiles[j] = sp

        emit_s(0)
        if nkc > 1:
            emit_s(1)
        if close_prev is not None:
            close_prev()
        rs = pool.tile([128, 2, QW], BF16, tag="rs", bufs=3, name="rs")
        rv = rs.rearrange("p a b -> p (a b)")
        for j in range(nkc):
            s_pair = s_tiles.pop(j)
            p_pair = pool.tile([128, 2, QW], BF16, tag="p", bufs=5,
                               name="p_pair")
            s_view = s_pair.rearrange("p a b -> p (a b)")
            p_view = p_pair.rearrange("p a b -> p (a b)")
            if dve_exp(j, hp):
                nc.vector.tensor_scalar(
                    p_view.bitcast(I16), s_view,
                    ss_t[:, j:j + 1], sb_t[:, j:j + 1], MUL, ADD)
            else:
                nc.scalar.activation(p_view, s_view, Exp,
                                     bias=kb_t[:, j:j + 1], scale=SCALE)
            for a in range(2):
                h = 2 * hp + a
                nc.tensor.matmul(
                    o_ps[a * 64:(a + 1) * 64, :],
                    v_nat[j][:, h * D:(h + 1) * D],
                    p_pair[:, a, :],
                    start=(j == 0), stop=(j == nkc - 1),
                    tile_position=(0, a * 64),
                    skip_group_check=True)
            if j + 2 < nkc:
                emit_s(j + 2)
            reng = nc.gpsimd if gps_rs(j, hp) else nc.vector
            if j == 0:
                reng.tensor_copy(rv, p_view)
            else:
                reng.tensor_add(rv, rv, p_view)
        # evacuate unnormalized O~ (bf16); normalize later
        oe = pool.tile([128, QW], BF16, tag="oe", bufs=7, name="oe")
        if hp % 2 == 0:
            nc.vector.tensor_copy(oe, o_ps)
        else:
            nc.scalar.copy(oe, o_ps)

        def close():
            dn_hp = misc_ps([2, QW], "dn_hp")
            for a in range(2):
                nc.tensor.matmul(
                    dn_hp, er[:, 2 * hp + a, 2 * hp:2 * hp + 2],
                    rs[:, a, :],
                    start=(a == 0), stop=(a == 1))
            nc.vector.reciprocal_approx_fast(
                out=rc_sb[:, hp, :], in_=dn_hp)

        return oe, close

    # ---------------- per-quarter tail ----------------
    ot = [[None] * QQ for _ in range(KCH)]

    def emit_qq_tail(qq, oe_l, rc_sb):
        rc_dram = dr.tile([2, KCH, QW], F32, tag="rc_dram", bufs=2,
                          name="rc_dram")
        nc.sync.dma_start(out=rc_dram, in_=rc_sb)
        for hp in range(KCH):
            rc_bc = pool.tile([128, QW], F32, tag="rc_bc", bufs=3,
                              name="rc_bc")
            for a in range(2):
                row = rc_dram[a:a + 1, hp:hp + 1, :]
                bc_ap = bass.AP(
                    tensor=row.tensor, offset=row.offset,
                    ap=[[0, 64]] + [list(p) for p in row.ap[1:]])
                nc.sync.dma_start(out=rc_bc[a * 64:(a + 1) * 64, :],
                                  in_=bc_ap)
            ott = pool.tile([128, QW], BF16, tag="ot", bufs=7, name="ott")
            nc.vector.tensor_mul(ott, oe_l[hp], rc_bc)
            ot[hp][qq] = ott
        tq = NCH // QQ

        def make_proj(t_i):
            def emit_proj():
                tl = (t_i % tq) * 128
                out_t = pool.tile([128, DIM], F32, tag="out_t", bufs=2,
                                  name="out_t")
                for half, (lo, hi) in enumerate(((0, 512), (512, DIM))):
                    pr_ps = misc_ps([128, 512], "pr_ps")
                    for c in range(KCH):
                        nc.tensor.matmul(
                            pr_ps[:, 0:hi - lo],
                            ot[c][qq][:, tl:tl + 128],
                            wp[c][:, lo:hi],
                            start=(c == 0), stop=(c == KCH - 1))
                    if (t_i + half) % 2 == 0:
                        nc.scalar.copy(out_t[:, lo:hi],
                                       pr_ps[:, 0:hi - lo])
                    else:
                        nc.vector.tensor_copy(out_t[:, lo:hi],
                                              pr_ps[:, 0:hi - lo])
                nc.sync.dma_start(
                    out=o_d[t_i * 128:(t_i + 1) * 128, :], in_=out_t)
            return emit_proj
        return [make_proj(t_i) for t_i in range(qq * tq, (qq + 1) * tq)]

    # ---------------- wavefront emission ----------------
    emit_kt(0, use_s=True)
    emit_qt(0, 0)
    pending_proj = []
    for qq in range(QQ):
        rc_sb = pool.tile([2, KCH, QW], F32, tag="rc_sb", bufs=1,
                          name="rc_sb")
        oe_l = []
        close = None
        for hp in range(KCH):
            if qq == 0 and hp + 1 < KCH:
                emit_kt(hp + 1)
                emit_qt(hp + 1, 0)
            oe, close = emit_attn(qq, hp, rc_sb, close)
            oe_l.append(oe)
            for _ in range(3):
                if pending_proj:
                    pending_proj.pop(0)()
        close()
        if qq == 1:
            emit_xt_half(1)
        if qq + 1 < QQ:
            for m in range(KCH):
                emit_qt(m, qq + 1)
        pending_proj.extend(emit_qq_tail(qq, oe_l, rc_sb))
    for fn in pending_proj:
        fn()

    pool.release()
    ps.release()
    dr.release()


_CACHE = {}


def _get_compiled(nkc):
    if nkc in _CACHE:
        return _CACHE[nkc]
    NK = nkc * 128
    nc = bacc.Bacc("TRN2", target_bir_lowering=False, debug=False,
                   num_devices=B)
    x_d = nc.dram_tensor("x", [N, DIM], F32, kind="ExternalInput").ap()
    ki_d = nc.dram_tensor("kidx", [NK], I32, kind="ExternalInput").ap()
    kb_d = nc.dram_tensor("kbias", [NK], F32, kind="ExternalInput").ap()
    ss_d = nc.dram_tensor("sscale", [NK], F32, kind="ExternalInput").ap()
    sb_d = nc.dram_tensor("sbias", [NK], F32, kind="ExternalInput").ap()
    wqkv_d = nc.dram_tensor("w_qkv", [DIM, 3 * DIM], F32,
                            kind="ExternalInput").ap()
    wp_d = nc.dram_tensor("w_proj", [DIM, DIM], F32,
                          kind="ExternalInput").ap()
    o_d = nc.dram_tensor("out", [N, DIM], F32, kind="ExternalOutput").ap()
    with tile.TileContext(nc) as tc:
        _build(nc, tc, (x_d, ki_d, kb_d, ss_d, sb_d, wqkv_d, wp_d, o_d),
               nkc)
    nc.compile()
    _CACHE[nkc] = nc
    return nc


def prep_run(x, mask, w_qkv, w_proj, b_proj):
    x = np.ascontiguousarray(np.asarray(x, dtype=np.float32))
    mask = np.ascontiguousarray(np.asarray(mask, dtype=np.int32))
    w_qkv = np.ascontiguousarray(np.asarray(w_qkv, dtype=np.float32))
    w_proj = np.ascontiguousarray(np.asarray(w_proj, dtype=np.float32))

    idxs = [np.flatnonzero(mask[b]).astype(np.int32) for b in range(B)]
    max_valid = max(len(i) for i in idxs)
    nkc = min(NCH, max(1, -(-max_valid // 128)))
    NK = nkc * 128
    kidx = np.zeros((B, NK), dtype=np.int32)
    kbias = np.full((B, NK), -1.0e30, dtype=np.float32)
    sscale = np.zeros((B, NK), dtype=np.float32)
    sbias = np.zeros((B, NK), dtype=np.float32)
    for b in range(B):
        n = len(idxs[b])
        kidx[b, :n] = idxs[b]
        kbias[b, :n] = 0.0
        sscale[b, :n] = SCHR_SCALE
        sbias[b, :n] = SCHR_BIAS

    nc = _get_compiled(nkc)
    in_maps = [
        {"x": x[b], "kidx": kidx[b], "kbias": kbias[b],
         "sscale": sscale[b], "sbias": sbias[b],
         "w_qkv": w_qkv, "w_proj": w_proj}
        for b in range(B)
    ]
    return nc, in_maps


def kernel(x, mask, w_qkv, w_proj, b_proj):
    nc, in_maps = prep_run(x, mask, w_qkv, w_proj, b_proj)
    b_proj = np.asarray(b_proj, dtype=np.float32)
    last_err = None
    for _ in range(3):
        try:
            res = run_bass_kernel_spmd(nc, in_maps, list(range(B))).results
            out = np.stack([res[b]["out"] for b in range(B)], axis=0)
            return out + b_proj
        except Exception as e:  # transient device hiccup: retry
            last_err = e
    raise last_err


# revision 16
# speedup vs baseline: 1.7867x; 1.0047x over previous
"""Trainium2 Bass kernel: masked multi-head self-attention block.

out = softmax_mask((x @ Wq) (x @ Wk)^T / sqrt(d)) (x @ Wv) @ Wp + b

Sharding: data-parallel over batch B=8 across the 8 NeuronCores (one
batch row per core); weights replicated; no collectives.

Key compaction: only valid key rows (mask==1) participate; indices are
computed on the host, rows gathered on-device via indirect DMA, padded
to a multiple of 128 (pad slots produce exp()==0).

v3 design (after the latency-bound v2 at 826us):
- Query dim processed in 512-wide quarters; S tiles are [128,2a,512]
  f32 (2 PSUM banks) holding BOTH heads of a pair, so exp runs as one
  [128,1024] instruction per key chunk and the dedicated "s" PSUM tag
  (2 slots) gives 2-key-chunk pipeline depth.  A separate "misc" tag
  serves QKV/proj/transpose/denominator matmuls so they never starve
  the S rotation (the v2 mistake), and "o" (2x 1-bank slots) holds the
  PV accumulators.
- exp engine split: ScalarE for 6 of 9 key chunks, DVE for chunks
  {3,5,7} via the one-op Schraudolph bit-trick (i16 = floor(S*(A/8)+B)
  is the bf16 exp bit pattern); per-partition scale/bias double as the
  pad mask.
- softmax row-sums: bf16 partial adds split DVE/GpSimd by key chunk;
  cross-partition reduction via two accumulating e_r matmuls per
  (quarter, head-pair) into a rotating [2,512] PSUM tile, evacuated to
  a per-quarter [12,512] table; one reciprocal + DRAM broadcast per
  quarter; O~ evacuated unnormalized (bf16) and scaled bf16*bf16.
- x transposes on PE in f32 with 8-wide batched evacuations; the xt
  stream borrows the idle "s" slots during lead-in while xct uses
  "misc", so the two streams run concurrently.
- b_proj is added on the host.
"""
import numpy as np

import concourse.bass as bass
import concourse.tile as tile
from concourse import bacc, mybir
from concourse.bass_utils import run_bass_kernel_spmd
from concourse.masks import make_identity

F32 = mybir.dt.float32
BF16 = mybir.dt.bfloat16
I16 = mybir.dt.int16
I32 = mybir.dt.int32

B, N, DIM = 8, 2048, 768
H, D = 12, 64
SCALE = D ** -0.5
NCH = N // 128        # 16 token chunks
KCH = DIM // 128      # 6 feature chunks
QQ = 4                # query quarters
QW = N // QQ          # 512 queries per quarter
Exp = mybir.ActivationFunctionType.Exp
MUL = mybir.AluOpType.mult
ADD = mybir.AluOpType.add

# Schraudolph constants (floor conversion semantics on DVE)
A16 = 128.0 / float(np.log(2.0))
SCHR_C = 5.1
SCHR_SCALE = A16 * SCALE
SCHR_BIAS = 127.0 * 128.0 - SCHR_C

def dve_exp(j, hp):
    return j == 5 or (j == 7 and hp % 2 == 0)


def gps_rs(j, hp):
    return j in (1, 4, 6)


def _build(nc, tc, aps, nkc):
    (x_d, ki_d, kb_d, ss_d, sb_d, wqkv_d, wp_d, o_d) = aps
    NK = nkc * 128

    pool = tc.alloc_tile_pool(name="sb", bufs=1)
    ps = tc.alloc_tile_pool(name="ps", bufs=1, space="PSUM")
    dr = tc.alloc_tile_pool(name="dr", bufs=1, space="DRAM")

    def s_ps_tile(shape, name):
        return ps.tile(shape, F32, tag="s", bufs=3, name=name)

    def misc_ps(shape, name):
        return ps.tile(shape, F32, tag="misc", bufs=1, name=name)

    # ---------------- constants ----------------
    ident = pool.tile([128, 128], F32, tag="ident")
    make_identity(nc, ident)
    er = pool.tile([128, 12, 12], BF16, tag="er")
    nc.vector.memset(er, 0.0)
    for r in range(12):
        nc.vector.memset(er[:, r, r:r + 1], 1.0)
    kb_t = pool.tile([128, nkc], F32, tag="kb")
    nc.sync.dma_start(out=kb_t, in_=kb_d.rearrange("(j p) -> p j", p=128))
    ss_t = pool.tile([128, nkc], F32, tag="ss")
    nc.sync.dma_start(out=ss_t, in_=ss_d.rearrange("(j p) -> p j", p=128))
    sb_t = pool.tile([128, nkc], F32, tag="sbv")
    nc.sync.dma_start(out=sb_t, in_=sb_d.rearrange("(j p) -> p j", p=128))
    ki_t = pool.tile([128, nkc], I32, tag="ki")
    nc.sync.dma_start(out=ki_t, in_=ki_d.rearrange("(j p) -> p j", p=128))

    # ---------------- persistent tiles ----------------
    xct = [pool.tile([128, NK], BF16, tag=f"xct{c}", name=f"xct{c}")
           for c in range(KCH)]
    kt = [pool.tile([128, NK], BF16, tag=f"kt{m}", name=f"kt{m}")
          for m in range(KCH)]
    v_nat = [pool.tile([128, DIM], BF16, tag=f"vn{t}", name=f"vn{t}")
             for t in range(nkc)]
    wqk = [pool.tile([128, 2 * DIM], BF16, tag=f"wqk{c}", name=f"wqk{c}")
           for c in range(KCH)]
    wv = [pool.tile([128, DIM], BF16, tag=f"wv{c}", name=f"wv{c}")
          for c in range(KCH)]
    wp = [pool.tile([128, DIM], BF16, tag=f"wp{c}", name=f"wp{c}")
          for c in range(KCH)]

    # ---------------- gathered X_c^T (all gathers first) ------------
    nb = 0
    while nb < nkc:
        bw = min(4, nkc - nb)
        xg_l = []
        for t_i in range(nb, nb + bw):
            xg = pool.tile([128, DIM], F32, tag="xg", bufs=8, name="xg")
            nc.gpsimd.indirect_dma_start(
                out=xg, out_offset=None, in_=x_d,
                in_offset=bass.IndirectOffsetOnAxis(
                    ap=ki_t[:, t_i:t_i + 1], axis=0))
            xg_l.append(xg)
        for c in range(KCH):
            tpb = misc_ps([128, 4, 128], "tpb")
            for i in range(bw):
                nc.tensor.transpose(
                    tpb[:, i, :], xg_l[i][:, c * 128:(c + 1) * 128],
                    ident)
            dst = xct[c][:, nb * 128:(nb + bw) * 128]
            src = tpb.rearrange("p a b -> p (a b)")[:, 0:bw * 128]
            if c % 2 == 0:
                nc.scalar.copy(dst, src)
            else:
                nc.vector.tensor_copy(dst, src)
        nb += bw

    # ---------------- weights: DMA + cast (V first) ----------------
    for c in range(KCH):
        wvs = pool.tile([128, DIM], F32, tag="wvstage", bufs=1,
                        name="wvs")
        nc.sync.dma_start(
            out=wvs, in_=wqkv_d[c * 128:(c + 1) * 128, 2 * DIM:3 * DIM])
        nc.gpsimd.tensor_copy(wv[c], wvs)

    # ---------------- X^T half tiles (8-batched transposes) ---------
    xt_half = {}

    def emit_xt_half(qh):
        xt_tiles = [pool.tile([128, N // 2], BF16, tag=f"xt{c}", bufs=1,
                              name=f"xt{c}h")
                    for c in range(KCH)]
        xt_half[qh] = xt_tiles
        xs = []
        for t in range(qh * 8, qh * 8 + 8):
            x_t = pool.tile([128, DIM], F32, tag="xg", bufs=8, name="x_t")
            nc.sync.dma_start(out=x_t, in_=x_d[t * 128:(t + 1) * 128, :])
            xs.append(x_t)
        for c in range(KCH):
            tpb = s_ps_tile([128, 8, 128], "tpb_s")
            for i in range(8):
                nc.tensor.transpose(
                    tpb[:, i, :], xs[i][:, c * 128:(c + 1) * 128], ident)
            dst = xt_tiles[c]
            src = tpb.rearrange("p a b -> p (a b)")
            if c % 2 == 0:
                nc.vector.tensor_copy(dst, src)
            else:
                nc.scalar.copy(dst, src)

    emit_xt_half(0)

    for c in range(KCH):
        wqs = pool.tile([128, 2 * DIM], F32, tag="wqstage", bufs=1,
                        name="wqs")
        nc.sync.dma_start(
            out=wqs, in_=wqkv_d[c * 128:(c + 1) * 128, 0:2 * DIM])
        nc.vector.tensor_copy(wqk[c], wqs)
        wps = pool.tile([128, DIM], F32, tag="wpstage", bufs=1,
                        name="wps")
        nc.sync.dma_start(out=wps, in_=wp_d[c * 128:(c + 1) * 128, :])
        nc.gpsimd.tensor_copy(wp[c], wps)

    # ---------------- V = X_c @ Wv ----------------
    for t_i in range(nkc):
        for half, (lo, hi) in enumerate(((0, 512), (512, DIM))):
            v_ps = (misc_ps([128, 512], "v_ps") if (t_i + half) % 2 == 0
                    else s_ps_tile([128, 512], "v_ps_s"))
            for c in range(KCH):
                nc.tensor.matmul(v_ps[:, 0:hi - lo],
                                 xct[c][:, t_i * 128:(t_i + 1) * 128],
                                 wv[c][:, lo:hi],
                                 start=(c == 0), stop=(c == KCH - 1))
            if (t_i + half) % 2 == 0:
                nc.scalar.copy(v_nat[t_i][:, lo:hi], v_ps[:, 0:hi - lo])
            else:
                nc.vector.tensor_copy(v_nat[t_i][:, lo:hi],
                                      v_ps[:, 0:hi - lo])

    # ---------------- Q^T (per quarter) and K^T ----------------
    qt_q = [[None] * KCH for _ in range(QQ)]

    def emit_qt(m, qq):
        xt_tiles = xt_half[qq // 2]
        xoff = (qq % 2) * QW
        qtile = pool.tile([128, QW], BF16, tag=f"qt{m}", bufs=2,
                          name=f"qt{m}q")
        qt_q[qq][m] = qtile
        mm_ps = misc_ps([128, QW], "qk_ps")
        for c in range(KCH):
            nc.tensor.matmul(
                mm_ps, wqk[c][:, m * 128:(m + 1) * 128],
                xt_tiles[c][:, xoff:xoff + QW],
                start=(c == 0), stop=(c == KCH - 1))
        if m % 2 == 0:
            nc.scalar.copy(qtile, mm_ps)
        else:
            nc.vector.tensor_copy(qtile, mm_ps)

    def emit_kt(m, use_s=False):
        wcol = DIM + m * 128
        off = 0
        while off < NK:
            w = min(512, NK - off)
            mm_ps = (s_ps_tile([128, QW], "qk_ps_s") if use_s
                     else misc_ps([128, QW], "qk_ps"))
            for c in range(KCH):
                nc.tensor.matmul(
                    mm_ps[:, 0:w], wqk[c][:, wcol:wcol + 128],
                    xct[c][:, off:off + w],
                    start=(c == 0), stop=(c == KCH - 1))
            if (off // 512) % 2 == 0:
                nc.scalar.copy(kt[m][:, off:off + w], mm_ps[:, 0:w])
            else:
                nc.vector.tensor_copy(kt[m][:, off:off + w],
                                      mm_ps[:, 0:w])
            off += w

    # ---------------- attention for one (qq, hp) ----------------
    def emit_attn(qq, hp, rc_sb, close_prev):
        kt_c = kt[hp]
        qt_c = qt_q[qq][hp]
        o_ps = ps.tile([128, QW], F32, tag="o", bufs=1, name="o_ps")
        s_tiles = {}

        def emit_s(j):
            sp = s_ps_tile([128, 2, QW], "s_pair")
            for a in range(2):
                r0 = a * 64
                nc.tensor.matmul(
                    sp[:, a, :],
                    kt_c[r0:r0 + 64, j * 128:(j + 1) * 128],
                    qt_c[r0:r0 + 64, :],
                    start=True, stop=True)
            s_tiles[j] = sp

        emit_s(0)
        if nkc > 1:
            emit_s(1)
        if close_prev is not None:
            close_prev()
        rs = pool.tile([128, 2, QW], BF16, tag="rs", bufs=4, name="rs")
        rv = rs.rearrange("p a b -> p (a b)")
        for j in range(nkc):
            s_pair = s_tiles.pop(j)
            p_pair = pool.tile([128, 2, QW], BF16, tag="p", bufs=6,
                               name="p_pair")
            s_view = s_pair.rearrange("p a b -> p (a b)")
            p_view = p_pair.rearrange("p a b -> p (a b)")
            if dve_exp(j, hp):
                nc.vector.tensor_scalar(
                    p_view.bitcast(I16), s_view,
                    ss_t[:, j:j + 1], sb_t[:, j:j + 1], MUL, ADD)
            else:
                nc.scalar.activation(p_view, s_view, Exp,
                                     bias=kb_t[:, j:j + 1], scale=SCALE)
            for a in range(2):
                h = 2 * hp + a
                nc.tensor.matmul(
                    o_ps[a * 64:(a + 1) * 64, :],
                    v_nat[j][:, h * D:(h + 1) * D],
                    p_pair[:, a, :],
                    start=(j == 0), stop=(j == nkc - 1),
                    tile_position=(0, a * 64),
                    skip_group_check=True)
            if j + 2 < nkc:
                emit_s(j + 2)
            reng = nc.gpsimd if gps_rs(j, hp) else nc.vector
            if j == 0:
                reng.tensor_copy(rv, p_view)
            else:
                reng.tensor_add(rv, rv, p_view)
        # evacuate unnormalized O~ (bf16); normalize later
        oe = pool.tile([128, QW], BF16, tag="oe", bufs=7, name="oe")
        if hp % 2 == 0:
            nc.vector.tensor_copy(oe, o_ps)
        else:
            nc.scalar.copy(oe, o_ps)

        def close():
            dn_hp = misc_ps([2, QW], "dn_hp")
            for a in range(2):
                nc.tensor.matmul(
                    dn_hp, er[:, 2 * hp + a, 2 * hp:2 * hp + 2],
                    rs[:, a, :],
                    start=(a == 0), stop=(a == 1))
            nc.vector.reciprocal_approx_fast(
                out=rc_sb[:, hp, :], in_=dn_hp)

        return oe, close

    # ---------------- per-quarter tail ----------------
    ot = [[None] * QQ for _ in range(KCH)]

    def emit_qq_tail(qq, oe_l, rc_sb):
        rc_dram = dr.tile([2, KCH, QW], F32, tag="rc_dram", bufs=2,
                          name="rc_dram")
        nc.sync.dma_start(out=rc_dram, in_=rc_sb)
        for hp in range(KCH):
            rc_bc = pool.tile([128, QW], F32, tag="rc_bc", bufs=3,
                              name="rc_bc")
            for a in range(2):
                row = rc_dram[a:a + 1, hp:hp + 1, :]
                bc_ap = bass.AP(
                    tensor=row.tensor, offset=row.offset,
                    ap=[[0, 64]] + [list(p) for p in row.ap[1:]])
                nc.sync.dma_start(out=rc_bc[a * 64:(a + 1) * 64, :],
                                  in_=bc_ap)
            ott = pool.tile([128, QW], BF16, tag="ot", bufs=7, name="ott")
            nc.vector.tensor_mul(ott, oe_l[hp], rc_bc)
            ot[hp][qq] = ott
        tq = NCH // QQ

        def make_proj(t_i):
            def emit_proj():
                tl = (t_i % tq) * 128
                out_t = pool.tile([128, DIM], F32, tag="out_t", bufs=2,
                                  name="out_t")
                for half, (lo, hi) in enumerate(((0, 512), (512, DIM))):
                    pr_ps = misc_ps([128, 512], "pr_ps")
                    for c in range(KCH):
                        nc.tensor.matmul(
                            pr_ps[:, 0:hi - lo],
                            ot[c][qq][:, tl:tl + 128],
                            wp[c][:, lo:hi],
                            start=(c == 0), stop=(c == KCH - 1))
                    if (t_i + half) % 2 == 0:
                        nc.scalar.copy(out_t[:, lo:hi],
                                       pr_ps[:, 0:hi - lo])
                    else:
                        nc.vector.tensor_copy(out_t[:, lo:hi],
                                              pr_ps[:, 0:hi - lo])
                nc.sync.dma_start(
                    out=o_d[t_i * 128:(t_i + 1) * 128, :], in_=out_t)
            return emit_proj
        return [make_proj(t_i) for t_i in range(qq * tq, (qq + 1) * tq)]

    # ---------------- wavefront emission ----------------
    emit_kt(0, use_s=True)
    emit_qt(0, 0)
    pending_proj = []
    for qq in range(QQ):
        rc_sb = pool.tile([2, KCH, QW], F32, tag="rc_sb", bufs=1,
                          name="rc_sb")
        oe_l = []
        close = None
        for hp in range(KCH):
            if qq == 0 and hp + 1 < KCH:
                emit_kt(hp + 1)
                emit_qt(hp + 1, 0)
            if qq + 1 < QQ:
                emit_qt(hp, qq + 1)
            oe, close = emit_attn(qq, hp, rc_sb, close)
            oe_l.append(oe)
            for _ in range(3):
                if pending_proj:
                    pending_proj.pop(0)()
        close()
        if qq == 0:
            emit_xt_half(1)
        pending_proj.extend(emit_qq_tail(qq, oe_l, rc_sb))
    for fn in pending_proj:
        fn()

    pool.release()
    ps.release()
    dr.release()


_CACHE = {}


def _get_compiled(nkc):
    if nkc in _CACHE:
        return _CACHE[nkc]
    NK = nkc * 128
    nc = bacc.Bacc("TRN2", target_bir_lowering=False, debug=False,
                   num_devices=B)
    x_d = nc.dram_tensor("x", [N, DIM], F32, kind="ExternalInput").ap()
    ki_d = nc.dram_tensor("kidx", [NK], I32, kind="ExternalInput").ap()
    kb_d = nc.dram_tensor("kbias", [NK], F32, kind="ExternalInput").ap()
    ss_d = nc.dram_tensor("sscale", [NK], F32, kind="ExternalInput").ap()
    sb_d = nc.dram_tensor("sbias", [NK], F32, kind="ExternalInput").ap()
    wqkv_d = nc.dram_tensor("w_qkv", [DIM, 3 * DIM], F32,
                            kind="ExternalInput").ap()
    wp_d = nc.dram_tensor("w_proj", [DIM, DIM], F32,
                          kind="ExternalInput").ap()
    o_d = nc.dram_tensor("out", [N, DIM], F32, kind="ExternalOutput").ap()
    with tile.TileContext(nc) as tc:
        _build(nc, tc, (x_d, ki_d, kb_d, ss_d, sb_d, wqkv_d, wp_d, o_d),
               nkc)
    nc.compile()
    _CACHE[nkc] = nc
    return nc


def prep_run(x, mask, w_qkv, w_proj, b_proj):
    x = np.ascontiguousarray(np.asarray(x, dtype=np.float32))
    mask = np.ascontiguousarray(np.asarray(mask, dtype=np.int32))
    w_qkv = np.ascontiguousarray(np.asarray(w_qkv, dtype=np.float32))
    w_proj = np.ascontiguousarray(np.asarray(w_proj, dtype=np.float32))

    idxs = [np.flatnonzero(mask[b]).astype(np.int32) for b in range(B)]
    max_valid = max(len(i) for i in idxs)
    nkc = min(NCH, max(1, -(-max_valid // 128)))
    NK = nkc * 128
    kidx = np.zeros((B, NK), dtype=np.int32)
    kbias = np.full((B, NK), -1.0e30, dtype=np.float32)
    sscale = np.zeros((B, NK), dtype=np.float32)
    sbias = np.zeros((B, NK), dtype=np.float32)
    for b in range(B):
        n = len(idxs[b])
        kidx[b, :n] = idxs[b]
        kbias[b, :n] = 0.0
        sscale[b, :n] = SCHR_SCALE
        sbias[b, :n] = SCHR_BIAS

    nc = _get_compiled(nkc)
    in_maps = [
        {"x": x[b], "kidx": kidx[b], "kbias": kbias[b],
         "sscale": sscale[b], "sbias": sbias[b],
         "w_qkv": w_qkv, "w_proj": w_proj}
        for b in range(B)
    ]
    return nc, in_maps


def kernel(x, mask, w_qkv, w_proj, b_proj):
    nc, in_maps = prep_run(x, mask, w_qkv, w_proj, b_proj)
    b_proj = np.asarray(b_proj, dtype=np.float32)
    last_err = None
    for _ in range(3):
        try:
            res = run_bass_kernel_spmd(nc, in_maps, list(range(B))).results
            out = np.stack([res[b]["out"] for b in range(B)], axis=0)
            return out + b_proj
        except Exception as e:  # transient device hiccup: retry
            last_err = e
    raise last_err


# revision 17
# speedup vs baseline: 1.7926x; 1.0033x over previous
"""Trainium2 Bass kernel: masked multi-head self-attention block.

out = softmax_mask((x @ Wq) (x @ Wk)^T / sqrt(d)) (x @ Wv) @ Wp + b

Sharding: data-parallel over batch B=8 across the 8 NeuronCores (one
batch row per core); weights replicated; no collectives.

Key compaction: only valid key rows (mask==1) participate; indices are
computed on the host, rows gathered on-device via indirect DMA, padded
to a multiple of 128 (pad slots produce exp()==0).

v3 design (after the latency-bound v2 at 826us):
- Query dim processed in 512-wide quarters; S tiles are [128,2a,512]
  f32 (2 PSUM banks) holding BOTH heads of a pair, so exp runs as one
  [128,1024] instruction per key chunk and the dedicated "s" PSUM tag
  (2 slots) gives 2-key-chunk pipeline depth.  A separate "misc" tag
  serves QKV/proj/transpose/denominator matmuls so they never starve
  the S rotation (the v2 mistake), and "o" (2x 1-bank slots) holds the
  PV accumulators.
- exp engine split: ScalarE for 6 of 9 key chunks, DVE for chunks
  {3,5,7} via the one-op Schraudolph bit-trick (i16 = floor(S*(A/8)+B)
  is the bf16 exp bit pattern); per-partition scale/bias double as the
  pad mask.
- softmax row-sums: bf16 partial adds split DVE/GpSimd by key chunk;
  cross-partition reduction via two accumulating e_r matmuls per
  (quarter, head-pair) into a rotating [2,512] PSUM tile, evacuated to
  a per-quarter [12,512] table; one reciprocal + DRAM broadcast per
  quarter; O~ evacuated unnormalized (bf16) and scaled bf16*bf16.
- x transposes on PE in f32 with 8-wide batched evacuations; the xt
  stream borrows the idle "s" slots during lead-in while xct uses
  "misc", so the two streams run concurrently.
- b_proj is added on the host.
"""
import numpy as np

import concourse.bass as bass
import concourse.tile as tile
from concourse import bacc, mybir
from concourse.bass_utils import run_bass_kernel_spmd
from concourse.masks import make_identity

F32 = mybir.dt.float32
BF16 = mybir.dt.bfloat16
I16 = mybir.dt.int16
I32 = mybir.dt.int32

B, N, DIM = 8, 2048, 768
H, D = 12, 64
SCALE = D ** -0.5
NCH = N // 128        # 16 token chunks
KCH = DIM // 128      # 6 feature chunks
QQ = 4                # query quarters
QW = N // QQ          # 512 queries per quarter
Exp = mybir.ActivationFunctionType.Exp
MUL = mybir.AluOpType.mult
ADD = mybir.AluOpType.add

# Schraudolph constants (floor conversion semantics on DVE)
A16 = 128.0 / float(np.log(2.0))
SCHR_C = 5.1
SCHR_SCALE = A16 * SCALE
SCHR_BIAS = 127.0 * 128.0 - SCHR_C

def dve_exp(j, hp):
    return j == 5 or (j == 7 and hp % 2 == 0)


def gps_rs(j, hp):
    return j in (1, 6)


def _build(nc, tc, aps, nkc):
    (x_d, ki_d, kb_d, ss_d, sb_d, wqkv_d, wp_d, o_d) = aps
    NK = nkc * 128

    pool = tc.alloc_tile_pool(name="sb", bufs=1)
    ps = tc.alloc_tile_pool(name="ps", bufs=1, space="PSUM")
    dr = tc.alloc_tile_pool(name="dr", bufs=1, space="DRAM")

    def s_ps_tile(shape, name):
        return ps.tile(shape, F32, tag="s", bufs=3, name=name)

    def misc_ps(shape, name):
        return ps.tile(shape, F32, tag="misc", bufs=1, name=name)

    # ---------------- constants ----------------
    ident = pool.tile([128, 128], F32, tag="ident")
    make_identity(nc, ident)
    er = pool.tile([128, 12, 12], BF16, tag="er")
    nc.vector.memset(er, 0.0)
    for r in range(12):
        nc.vector.memset(er[:, r, r:r + 1], 1.0)
    kb_t = pool.tile([128, nkc], F32, tag="kb")
    nc.sync.dma_start(out=kb_t, in_=kb_d.rearrange("(j p) -> p j", p=128))
    ss_t = pool.tile([128, nkc], F32, tag="ss")
    nc.sync.dma_start(out=ss_t, in_=ss_d.rearrange("(j p) -> p j", p=128))
    sb_t = pool.tile([128, nkc], F32, tag="sbv")
    nc.sync.dma_start(out=sb_t, in_=sb_d.rearrange("(j p) -> p j", p=128))
    ki_t = pool.tile([128, nkc], I32, tag="ki")
    nc.sync.dma_start(out=ki_t, in_=ki_d.rearrange("(j p) -> p j", p=128))

    # ---------------- persistent tiles ----------------
    xct = [pool.tile([128, NK], BF16, tag=f"xct{c}", name=f"xct{c}")
           for c in range(KCH)]
    kt = [pool.tile([128, NK], BF16, tag=f"kt{m}", name=f"kt{m}")
          for m in range(KCH)]
    v_nat = [pool.tile([128, DIM], BF16, tag=f"vn{t}", name=f"vn{t}")
             for t in range(nkc)]
    wqk = [pool.tile([128, 2 * DIM], BF16, tag=f"wqk{c}", name=f"wqk{c}")
           for c in range(KCH)]
    wv = [pool.tile([128, DIM], BF16, tag=f"wv{c}", name=f"wv{c}")
          for c in range(KCH)]
    wp = [pool.tile([128, DIM], BF16, tag=f"wp{c}", name=f"wp{c}")
          for c in range(KCH)]

    # ---------------- gathered X_c^T (all gathers first) ------------
    nb = 0
    while nb < nkc:
        bw = min(4, nkc - nb)
        xg_l = []
        for t_i in range(nb, nb + bw):
            xg = pool.tile([128, DIM], F32, tag="xg", bufs=8, name="xg")
            nc.gpsimd.indirect_dma_start(
                out=xg, out_offset=None, in_=x_d,
                in_offset=bass.IndirectOffsetOnAxis(
                    ap=ki_t[:, t_i:t_i + 1], axis=0))
            xg_l.append(xg)
        for c in range(KCH):
            tpb = misc_ps([128, 4, 128], "tpb")
            for i in range(bw):
                nc.tensor.transpose(
                    tpb[:, i, :], xg_l[i][:, c * 128:(c + 1) * 128],
                    ident)
            dst = xct[c][:, nb * 128:(nb + bw) * 128]
            src = tpb.rearrange("p a b -> p (a b)")[:, 0:bw * 128]
            if c % 2 == 0:
                nc.scalar.copy(dst, src)
            else:
                nc.vector.tensor_copy(dst, src)
        nb += bw

    # ---------------- weights: DMA + cast (V first) ----------------
    for c in range(KCH):
        wvs = pool.tile([128, DIM], F32, tag="wvstage", bufs=1,
                        name="wvs")
        nc.sync.dma_start(
            out=wvs, in_=wqkv_d[c * 128:(c + 1) * 128, 2 * DIM:3 * DIM])
        nc.gpsimd.tensor_copy(wv[c], wvs)

    # ---------------- X^T half tiles (8-batched transposes) ---------
    xt_half = {}

    def emit_xt_half(qh):
        xt_tiles = [pool.tile([128, N // 2], BF16, tag=f"xt{c}", bufs=1,
                              name=f"xt{c}h")
                    for c in range(KCH)]
        xt_half[qh] = xt_tiles
        xs = []
        for t in range(qh * 8, qh * 8 + 8):
            x_t = pool.tile([128, DIM], F32, tag="xg", bufs=8, name="x_t")
            nc.sync.dma_start(out=x_t, in_=x_d[t * 128:(t + 1) * 128, :])
            xs.append(x_t)
        for c in range(KCH):
            tpb = s_ps_tile([128, 8, 128], "tpb_s")
            for i in range(8):
                nc.tensor.transpose(
                    tpb[:, i, :], xs[i][:, c * 128:(c + 1) * 128], ident)
            dst = xt_tiles[c]
            src = tpb.rearrange("p a b -> p (a b)")
            if c % 2 == 0:
                nc.vector.tensor_copy(dst, src)
            else:
                nc.scalar.copy(dst, src)

    for c in range(KCH):
        wqs = pool.tile([128, 2 * DIM], F32, tag="wqstage", bufs=1,
                        name="wqs")
        nc.sync.dma_start(
            out=wqs, in_=wqkv_d[c * 128:(c + 1) * 128, 0:2 * DIM])
        nc.vector.tensor_copy(wqk[c], wqs)
        wps = pool.tile([128, DIM], F32, tag="wpstage", bufs=1,
                        name="wps")
        nc.sync.dma_start(out=wps, in_=wp_d[c * 128:(c + 1) * 128, :])
        nc.gpsimd.tensor_copy(wp[c], wps)

    emit_xt_half(0)

    # ---------------- V = X_c @ Wv ----------------
    for t_i in range(nkc):
        for half, (lo, hi) in enumerate(((0, 512), (512, DIM))):
            v_ps = (misc_ps([128, 512], "v_ps") if (t_i + half) % 2 == 0
                    else s_ps_tile([128, 512], "v_ps_s"))
            for c in range(KCH):
                nc.tensor.matmul(v_ps[:, 0:hi - lo],
                                 xct[c][:, t_i * 128:(t_i + 1) * 128],
                                 wv[c][:, lo:hi],
                                 start=(c == 0), stop=(c == KCH - 1))
            if (t_i + half) % 2 == 0:
                nc.scalar.copy(v_nat[t_i][:, lo:hi], v_ps[:, 0:hi - lo])
            else:
                nc.vector.tensor_copy(v_nat[t_i][:, lo:hi],
                                      v_ps[:, 0:hi - lo])

    # ---------------- Q^T (per quarter) and K^T ----------------
    qt_q = [[None] * KCH for _ in range(QQ)]

    def emit_qt(m, qq):
        xt_tiles = xt_half[qq // 2]
        xoff = (qq % 2) * QW
        qtile = pool.tile([128, QW], BF16, tag=f"qt{m}", bufs=2,
                          name=f"qt{m}q")
        qt_q[qq][m] = qtile
        mm_ps = misc_ps([128, QW], "qk_ps")
        for c in range(KCH):
            nc.tensor.matmul(
                mm_ps, wqk[c][:, m * 128:(m + 1) * 128],
                xt_tiles[c][:, xoff:xoff + QW],
                start=(c == 0), stop=(c == KCH - 1))
        if m % 2 == 0:
            nc.scalar.copy(qtile, mm_ps)
        else:
            nc.vector.tensor_copy(qtile, mm_ps)

    def emit_kt(m, use_s=False):
        wcol = DIM + m * 128
        off = 0
        while off < NK:
            w = min(512, NK - off)
            mm_ps = (s_ps_tile([128, QW], "qk_ps_s") if use_s
                     else misc_ps([128, QW], "qk_ps"))
            for c in range(KCH):
                nc.tensor.matmul(
                    mm_ps[:, 0:w], wqk[c][:, wcol:wcol + 128],
                    xct[c][:, off:off + w],
                    start=(c == 0), stop=(c == KCH - 1))
            if (off // 512) % 2 == 0:
                nc.scalar.copy(kt[m][:, off:off + w], mm_ps[:, 0:w])
            else:
                nc.vector.tensor_copy(kt[m][:, off:off + w],
                                      mm_ps[:, 0:w])
            off += w

    # ---------------- attention for one (qq, hp) ----------------
    def emit_attn(qq, hp, rc_sb, close_prev):
        kt_c = kt[hp]
        qt_c = qt_q[qq][hp]
        o_ps = ps.tile([128, QW], F32, tag="o", bufs=1, name="o_ps")
        s_tiles = {}

        def emit_s(j):
            sp = s_ps_tile([128, 2, QW], "s_pair")
            for a in range(2):
                r0 = a * 64
                nc.tensor.matmul(
                    sp[:, a, :],
                    kt_c[r0:r0 + 64, j * 128:(j + 1) * 128],
                    qt_c[r0:r0 + 64, :],
                    start=True, stop=True)
            s_tiles[j] = sp

        emit_s(0)
        if nkc > 1:
            emit_s(1)
        if close_prev is not None:
            close_prev()
        rs = pool.tile([128, 2, QW], BF16, tag="rs", bufs=4, name="rs")
        rv = rs.rearrange("p a b -> p (a b)")
        for j in range(nkc):
            s_pair = s_tiles.pop(j)
            p_pair = pool.tile([128, 2, QW], BF16, tag="p", bufs=6,
                               name="p_pair")
            s_view = s_pair.rearrange("p a b -> p (a b)")
            p_view = p_pair.rearrange("p a b -> p (a b)")
            if dve_exp(j, hp):
                nc.vector.tensor_scalar(
                    p_view.bitcast(I16), s_view,
                    ss_t[:, j:j + 1], sb_t[:, j:j + 1], MUL, ADD)
            else:
                nc.scalar.activation(p_view, s_view, Exp,
                                     bias=kb_t[:, j:j + 1], scale=SCALE)
            for a in range(2):
                h = 2 * hp + a
                nc.tensor.matmul(
                    o_ps[a * 64:(a + 1) * 64, :],
                    v_nat[j][:, h * D:(h + 1) * D],
                    p_pair[:, a, :],
                    start=(j == 0), stop=(j == nkc - 1),
                    tile_position=(0, a * 64),
                    skip_group_check=True)
            if j + 2 < nkc:
                emit_s(j + 2)
            reng = nc.gpsimd if gps_rs(j, hp) else nc.vector
            if j == 0:
                reng.tensor_copy(rv, p_view)
            else:
                reng.tensor_add(rv, rv, p_view)
        # evacuate unnormalized O~ (bf16); normalize later
        oe = pool.tile([128, QW], BF16, tag="oe", bufs=7, name="oe")
        if hp % 2 == 0:
            nc.vector.tensor_copy(oe, o_ps)
        else:
            nc.scalar.copy(oe, o_ps)

        def close():
            dn_hp = misc_ps([2, QW], "dn_hp")
            for a in range(2):
                nc.tensor.matmul(
                    dn_hp, er[:, 2 * hp + a, 2 * hp:2 * hp + 2],
                    rs[:, a, :],
                    start=(a == 0), stop=(a == 1))
            nc.vector.reciprocal_approx_fast(
                out=rc_sb[:, hp, :], in_=dn_hp)

        return oe, close

    # ---------------- per-quarter tail ----------------
    ot = [[None] * QQ for _ in range(KCH)]

    def emit_qq_tail(qq, oe_l, rc_sb):
        rc_dram = dr.tile([2, KCH, QW], F32, tag="rc_dram", bufs=2,
                          name="rc_dram")
        nc.sync.dma_start(out=rc_dram, in_=rc_sb)
        for hp in range(KCH):
            rc_bc = pool.tile([128, QW], F32, tag="rc_bc", bufs=3,
                              name="rc_bc")
            for a in range(2):
                row = rc_dram[a:a + 1, hp:hp + 1, :]
                bc_ap = bass.AP(
                    tensor=row.tensor, offset=row.offset,
                    ap=[[0, 64]] + [list(p) for p in row.ap[1:]])
                nc.sync.dma_start(out=rc_bc[a * 64:(a + 1) * 64, :],
                                  in_=bc_ap)
            ott = pool.tile([128, QW], BF16, tag="ot", bufs=7, name="ott")
            nc.vector.tensor_mul(ott, oe_l[hp], rc_bc)
            ot[hp][qq] = ott
        tq = NCH // QQ

        def make_proj(t_i):
            def emit_proj():
                tl = (t_i % tq) * 128
                out_t = pool.tile([128, DIM], F32, tag="out_t", bufs=2,
                                  name="out_t")
                for half, (lo, hi) in enumerate(((0, 512), (512, DIM))):
                    pr_ps = misc_ps([128, 512], "pr_ps")
                    for c in range(KCH):
                        nc.tensor.matmul(
                            pr_ps[:, 0:hi - lo],
                            ot[c][qq][:, tl:tl + 128],
                            wp[c][:, lo:hi],
                            start=(c == 0), stop=(c == KCH - 1))
                    if (t_i + half) % 2 == 0:
                        nc.scalar.copy(out_t[:, lo:hi],
                                       pr_ps[:, 0:hi - lo])
                    else:
                        nc.vector.tensor_copy(out_t[:, lo:hi],
                                              pr_ps[:, 0:hi - lo])
                nc.sync.dma_start(
                    out=o_d[t_i * 128:(t_i + 1) * 128, :], in_=out_t)
            return emit_proj
        return [make_proj(t_i) for t_i in range(qq * tq, (qq + 1) * tq)]

    # ---------------- wavefront emission ----------------
    emit_kt(0, use_s=True)
    emit_qt(0, 0)
    pending_proj = []
    for qq in range(QQ):
        rc_sb = pool.tile([2, KCH, QW], F32, tag="rc_sb", bufs=1,
                          name="rc_sb")
        oe_l = []
        close = None
        for hp in range(KCH):
            if qq == 0 and hp + 1 < KCH:
                emit_kt(hp + 1)
                emit_qt(hp + 1, 0)
            if qq + 1 < QQ:
                emit_qt(hp, qq + 1)
            oe, close = emit_attn(qq, hp, rc_sb, close)
            oe_l.append(oe)
            for _ in range(3):
                if pending_proj:
                    pending_proj.pop(0)()
        close()
        if qq == 0:
            emit_xt_half(1)
        pending_proj.extend(emit_qq_tail(qq, oe_l, rc_sb))
    for fn in pending_proj:
        fn()

    pool.release()
    ps.release()
    dr.release()


_CACHE = {}


def _get_compiled(nkc):
    if nkc in _CACHE:
        return _CACHE[nkc]
    NK = nkc * 128
    nc = bacc.Bacc("TRN2", target_bir_lowering=False, debug=False,
                   num_devices=B)
    x_d = nc.dram_tensor("x", [N, DIM], F32, kind="ExternalInput").ap()
    ki_d = nc.dram_tensor("kidx", [NK], I32, kind="ExternalInput").ap()
    kb_d = nc.dram_tensor("kbias", [NK], F32, kind="ExternalInput").ap()
    ss_d = nc.dram_tensor("sscale", [NK], F32, kind="ExternalInput").ap()
    sb_d = nc.dram_tensor("sbias", [NK], F32, kind="ExternalInput").ap()
    wqkv_d = nc.dram_tensor("w_qkv", [DIM, 3 * DIM], F32,
                            kind="ExternalInput").ap()
    wp_d = nc.dram_tensor("w_proj", [DIM, DIM], F32,
                          kind="ExternalInput").ap()
    o_d = nc.dram_tensor("out", [N, DIM], F32, kind="ExternalOutput").ap()
    with tile.TileContext(nc) as tc:
        _build(nc, tc, (x_d, ki_d, kb_d, ss_d, sb_d, wqkv_d, wp_d, o_d),
               nkc)
    nc.compile()
    _CACHE[nkc] = nc
    return nc


def prep_run(x, mask, w_qkv, w_proj, b_proj):
    x = np.ascontiguousarray(np.asarray(x, dtype=np.float32))
    mask = np.ascontiguousarray(np.asarray(mask, dtype=np.int32))
    w_qkv = np.ascontiguousarray(np.asarray(w_qkv, dtype=np.float32))
    w_proj = np.ascontiguousarray(np.asarray(w_proj, dtype=np.float32))

    idxs = [np.flatnonzero(mask[b]).astype(np.int32) for b in range(B)]
    max_valid = max(len(i) for i in idxs)
    nkc = min(NCH, max(1, -(-max_valid // 128)))
    NK = nkc * 128
    kidx = np.zeros((B, NK), dtype=np.int32)
    kbias = np.full((B, NK), -1.0e30, dtype=np.float32)
    sscale = np.zeros((B, NK), dtype=np.float32)
    sbias = np.zeros((B, NK), dtype=np.float32)
    for b in range(B):
        n = len(idxs[b])
        kidx[b, :n] = idxs[b]
        kbias[b, :n] = 0.0
        sscale[b, :n] = SCHR_SCALE
        sbias[b, :n] = SCHR_BIAS

    nc = _get_compiled(nkc)
    in_maps = [
        {"x": x[b], "kidx": kidx[b], "kbias": kbias[b],
         "sscale": sscale[b], "sbias": sbias[b],
         "w_qkv": w_qkv, "w_proj": w_proj}
        for b in range(B)
    ]
    return nc, in_maps


def kernel(x, mask, w_qkv, w_proj, b_proj):
    nc, in_maps = prep_run(x, mask, w_qkv, w_proj, b_proj)
    b_proj = np.asarray(b_proj, dtype=np.float32)
    last_err = None
    for _ in range(3):
        try:
            res = run_bass_kernel_spmd(nc, in_maps, list(range(B))).results
            out = np.stack([res[b]["out"] for b in range(B)], axis=0)
            return out + b_proj
        except Exception as e:  # transient device hiccup: retry
            last_err = e
    raise last_err
